# revision 33
# baseline (speedup 1.0000x reference)
"""Attention-LSTM decoder (B=32, T=1000, S=100, D=512, A=1024, H=1024,
E=640, V=10240, P=1024) on 8 trn2 NeuronCores.

Sharding: data-parallel over batch, 4 batches per core (one per "slot").
Batches are sorted by enc_seq_len; slot j holds ranks [j*8:(j+1)*8] so the
padded time extent Tp[j] (multiple of 128) is shared by all 8 cores and the
SPMD graph is identical across cores.

v3 design (vs. v2 baseline):
  - attention slots split into group A = slots {0,1,2} and group B = {3}.
    Group A's softmax / transpose / ctx-matvec tail executes underneath
    group B's tanh stretch; only B's short tail is serial.
  - tanh merged per (group, a-chunk): s_t/wfb is folded into the z-add via
    scalar_tensor_tensor with the per-partition scalar read directly from
    the s PSUM tile (host pre-divides W_s rows by wfb), so one ACT
    instruction covers all slots of a group.
  - softmax: exp -> one STT that applies the {0,1} mask AND emits the row
    sums via accum_out (no tensor_reduce); w*finv fused the same way.
  - gates accumulate in a single PSUM tile: W_hh part prefetched during the
    previous stretch, ctx part + xemb (via identity matmul) appended, and
    the activations read PSUM directly.
  - LSTM pointwise lowered to 5 STT + 1 TT + 1 TS using 2*sigmoid(x) =
    tanh(x/2) + 1; zoneout blends use pre-scaled c05/h05 computed during
    the previous stretch.
"""
import sys

sys.path.insert(0, "/opt/trn_rl_repo")

import os
import numpy as np
import ml_dtypes
from contextlib import ExitStack

import concourse.bass as bass
import concourse.tile as tile
import concourse.mybir as mybir
from concourse import bacc
from concourse.masks import make_identity

DT = mybir.dt
F32 = DT.float32
BF16 = DT.bfloat16
FP8 = DT.float8e4
AF = mybir.ActivationFunctionType
ALU = mybir.AluOpType
ET = mybir.EngineType

B, T, S = 32, 1000, 100
D, A, H, E, V, RO = 512, 1024, 1024, 640, 10240, 1024
ZH, ZC = 0.05, 0.15
NCORE = 8
BL = B // NCORE          # 4 batches (slots) per core
NS = S * BL              # 400 step-batch columns
GC = 4 * H // 128        # 32 gate chunks
HC = H // 128            # 8
AC = A // 128            # 8
DC = D // 128            # 4
EC = E // 128            # 5
ROC = RO // 2 // 128     # 4 chunks per maxout half
VC = V // 128            # 80 vocab chunks
XROC = (H + E + D) // 128  # 17 readout K-chunks

USE_FP8 = os.environ.get("KBFP8", "1") != "0"
SW = 64.0 if USE_FP8 else 1.0    # weight scale
SX = 16.0 if USE_FP8 else 1.0    # moving (h/ctx) scale
SG = SW * SX                      # psum scale for gates
WDT = FP8 if USE_FP8 else BF16

GA = (0, 1, 2)  # group A slots
GB = (3,)       # group B slots

bf16 = ml_dtypes.bfloat16
f8 = ml_dtypes.float8_e4m3
LAST_EXEC_NS = None
LAST_OUTS = None
LAST_META = None


def _bf(a):
    return np.ascontiguousarray(np.asarray(a, dtype=np.float32)).astype(bf16)


def _w8(a):
    a = np.asarray(a, dtype=np.float32) * SW
    return np.ascontiguousarray(a).astype(f8 if USE_FP8 else bf16)


# gate-permutation: reference gate order is [i|f|g|o]; we reorder rows to
# [i|f|o|g] so the three sigmoids are contiguous.
def _gate_perm():
    idx = np.arange(4 * H)
    return np.concatenate([idx[0:2 * H], idx[3 * H:4 * H], idx[2 * H:3 * H]])


def build_nc(Tp, debug=False):
    TC = [t // 128 for t in Tp]
    TCmax = max(TC)
    offA = [0, Tp[0], Tp[0] + Tp[1]]       # segment offsets in group-A tiles
    LA = Tp[0] + Tp[1] + Tp[2]
    LB = Tp[3]
    TpA = Tp[0]                            # group-A col extent (max of group)
    nc = bacc.Bacc("TRN2", target_bir_lowering=False)

    def param(name, shape, dt=BF16):
        return nc.declare_dram_parameter(name, list(shape), dt, isOutput=False)

    enc_td = [param(f"enc_td{j}", [Tp[j], D]) for j in range(BL)]
    encT = [param(f"encT{j}", [D, Tp[j]]) for j in range(BL)]
    embT_d = param("embT", [E, NS])
    W_combT_d = param("W_combT", [D + H, 4 * H], WDT)
    W_ih_embT_d = param("W_ih_embT", [E, 4 * H])
    W_encT_d = param("W_encT", [D, A])
    W_sT_d = param("W_sT", [H, A])                 # bf16: W_s / wfb
    wfert_col_d = param("wfert_col", [128, DC])
    vT_col_d = param("vT_col", [128, AC])
    wfb_colf_d = param("wfb_colf", [128, AC], F32)
    b_enc_col_d = param("b_enc_col", [128, AC], F32)
    b_comb_d = param("b_comb", [128, GC], F32)     # pre-scaled by SG on host
    mask01_d = param("mask01", [BL, 1024])         # {0,1} rows
    W_roT_e_d = param("W_roT_e", [H + E + D, RO // 2])
    W_roT_o_d = param("W_roT_o", [H + E + D, RO // 2])
    b_ro_e_d = param("b_ro_e", [128, ROC], F32)
    b_ro_o_d = param("b_ro_o", [128, ROC], F32)
    W_outT_d = param("W_outT", [RO // 2, V])
    b_out_d = param("b_out_col", [128, VC], F32)
    out_d = nc.declare_dram_parameter("out", [V, NS], F32, isOutput=True)

    qd = nc.dram_tensor("qd", [1, BL * 1024], BF16)
    dbg_d = nc.declare_dram_parameter("dbgt", [128, 1088], BF16,
                                      isOutput=True) \
        if os.environ.get("KBDBG") else None
    dbg2_d = nc.declare_dram_parameter("dbgt2", [128, 576], F32,
                                       isOutput=True) \
        if os.environ.get("KBDBG") else None
    hstk_d = nc.dram_tensor("hstk", [H, NS], BF16)
    cstk_d = nc.dram_tensor("cstk", [D, NS], BF16)
    xembT_d = nc.dram_tensor("xembT", [4 * H, NS + BL], BF16)

    with ExitStack() as ctx:
        tc = ctx.enter_context(tile.TileContext(nc))

        # ---------------- persistent pools ----------------
        persist = ctx.enter_context(tc.tile_pool(name="persist", bufs=1))
        ident = persist.tile([128, 128], BF16)
        make_identity(nc, ident[:])
        vT_col = persist.tile([128, AC], BF16)
        nc.sync.dma_start(vT_col[:], vT_col_d[:, :])
        wfb_colf = persist.tile([128, AC], F32)
        nc.sync.dma_start(wfb_colf[:], wfb_colf_d[:, :])
        wfert_col = persist.tile([128, DC], BF16)
        nc.sync.dma_start(wfert_col[:], wfert_col_d[:, :])
        b_enc_col = persist.tile([128, AC], F32)
        nc.sync.dma_start(b_enc_col[:], b_enc_col_d[:, :])
        b_comb = persist.tile([128, GC], F32)
        nc.sync.dma_start(b_comb[:], b_comb_d[:, :])
        mask01 = persist.tile([128, 1024], BF16)
        nc.vector.memset(mask01[:], 0.0)
        # col 0 = 1 on every row so dead-row softmax sums stay finite
        nc.vector.memset(mask01[:, 0:1], 1.0)
        for j in range(BL):
            nc.sync.dma_start(mask01[32 * j:32 * j + 1, :], mask01_d[j:j + 1, :])

        h_bf = persist.tile([128, HC * BL], BF16)
        h_q = persist.tile([128, HC * BL], WDT)
        c_st = persist.tile([128, HC * BL], F32)
        c05 = persist.tile([128, HC * BL], F32)
        h05 = persist.tile([128, HC * BL], F32)
        ctxT_sb = persist.tile([128, DC * BL], BF16)
        ctx_q = persist.tile([128, DC * BL], WDT)
        accum_bf = persist.tile([128, 1024], BF16)  # rows {0,32,64,96}
        w_att = persist.tile([128, 1024], BF16)
        finv = persist.tile([128, 1024], BF16)     # rows {0,32,64,96}, x0.5
        wts = persist.tile([128, TCmax * BL], BF16)
        zrow = persist.tile([1, 128], BF16)
        nc.vector.memset(zrow[:], 0.0)
        zpad = persist.tile([128, GC * BL], BF16)
        nc.vector.memset(zpad[:], 0.0)
        nc.sync.dma_start(
            xembT_d[:, NS:NS + BL].rearrange("(c p) b -> p c b", p=128),
            zpad[:])
        for t_ in (h_bf, h_q, c_st, c05, h05, ctxT_sb, ctx_q, accum_bf,
                   w_att, finv, wts):
            nc.vector.memset(t_[:], 0.0)

        inner = ctx.enter_context(ExitStack())
        e_pool = inner.enter_context(tc.tile_pool(name="e", bufs=1))
        e_A = [e_pool.tile([128, LA], BF16, name=f"eA{a}", tag=f"eA{a}")
               for a in range(AC)]
        e_B = [e_pool.tile([128, LB], BF16, name=f"eB{a}", tag=f"eB{a}")
               for a in range(AC)]

        trash_holder = []

        def pe_touch(ap):
            # phase-scoped trash tile (pre/post only; fp8 touches are no-ops)
            if ap.dtype not in (BF16,) or not trash_holder:
                return
            trash_ps = trash_holder[0]
            p = ap.shape[0]
            nc.tensor.transpose(trash_ps[0:min(ap.shape[1], 128), 0:p],
                                ap[:, 0:min(ap.shape[1], 128)], ident[0:p, 0:p])

        # ============ PRECOMPUTE PHASE ============
        with ExitStack() as pre:
            pre_sb = pre.enter_context(tc.tile_pool(name="pre_sb", bufs=1))
            pre_st = pre.enter_context(tc.tile_pool(name="pre_st", bufs=2))
            pre_ps = pre.enter_context(tc.tile_pool(name="pre_ps", bufs=1,
                                                    space="PSUM"))
            trash_holder.append(pre_ps.tile([128, 128], BF16, name="trash_pre"))

            W_encT = [pre_sb.tile([128, A], BF16, name=f"wenc{k}", tag=f"we{k}")
                      for k in range(DC)]
            for k in range(DC):
                nc.sync.dma_start(W_encT[k][:], W_encT_d[k * 128:(k + 1) * 128, :])
            pe_touch(W_encT[0][:, 0:128])

            for j in range(BL):
                ercs = [pre_st.tile([128, Tp[j]], BF16, name=f"erc{j}{k}",
                                    tag=f"erc{k}") for k in range(DC)]
                for k in range(DC):
                    nc.sync.dma_start(ercs[k][:], encT[j][k * 128:(k + 1) * 128, :])
                    pe_touch(ercs[k][:, 0:128])
                for a in range(AC):
                    pe2 = pre_ps.tile([128, 1024], F32, name="pe_e2", tag="pe_e2")
                    for k in range(DC):
                        for n0 in range(0, Tp[j], 512):
                            n1 = min(n0 + 512, Tp[j])
                            nc.tensor.matmul(pe2[:, n0:n1],
                                             W_encT[k][:, a * 128:(a + 1) * 128],
                                             ercs[k][:, n0:n1],
                                             start=(k == 0), stop=(k == DC - 1))
                    if j in GA:
                        dst = e_A[a][:, offA[j]:offA[j] + Tp[j]]
                    else:
                        dst = e_B[a][:, 0:Tp[j]]
                    nc.scalar.activation(dst, pe2[:, 0:Tp[j]],
                                         AF.Identity,
                                         bias=b_enc_col[:, a:a + 1], scale=1.0)
                pf = pre_ps.tile([1, 1024], F32, name="pf", tag="pf")
                for k in range(DC):
                    for n0 in range(0, Tp[j], 512):
                        n1 = min(n0 + 512, Tp[j])
                        nc.tensor.matmul(pf[0:1, n0:n1], wfert_col[:, k:k + 1],
                                         ercs[k][:, n0:n1],
                                         start=(k == 0), stop=(k == DC - 1))
                # finv = 0.5*sigmoid(x) = 0.25*tanh(0.5x) + 0.25  (no table sw)
                fstage = pre_st.tile([1, 1024], F32, name="fstage", tag="fstage")
                nc.scalar.activation(fstage[0:1, 0:Tp[j]], pf[0:1, 0:Tp[j]],
                                     AF.Tanh, scale=0.5)
                fst2 = pre_st.tile([1, 1024], BF16, name="fst2", tag="fst2")
                nc.vector.tensor_scalar(fst2[0:1, 0:Tp[j]], fstage[0:1, 0:Tp[j]],
                                        0.25, 0.25, ALU.mult, ALU.add)
                nc.sync.dma_start(finv[32 * j:32 * j + 1, 0:Tp[j]],
                                  fst2[0:1, 0:Tp[j]])

            embT_sb = [pre_sb.tile([128, NS], BF16, name=f"embs{k}", tag=f"em{k}")
                       for k in range(EC)]
            for k in range(EC):
                nc.sync.dma_start(embT_sb[k][:], embT_d[k * 128:(k + 1) * 128, :])
            W_ie = [pre_sb.tile([128, 4 * H], BF16, name=f"wie{k}", tag=f"wi{k}")
                    for k in range(EC)]
            for k in range(EC):
                nc.sync.dma_start(W_ie[k][:], W_ih_embT_d[k * 128:(k + 1) * 128, :])
            pe_touch(W_ie[0][:, 0:128])
            pe_touch(embT_sb[0][:, 0:128])
            for g in range(GC):
                px = pre_ps.tile([128, NS], F32, name="px", tag="pe_e2")
                for k in range(EC):
                    nc.tensor.matmul(px[:], W_ie[k][:, g * 128:(g + 1) * 128],
                                     embT_sb[k][:], start=(k == 0),
                                     stop=(k == EC - 1))
                # xemb scaled by SG, bias pre-scaled on host
                stg = pre_st.tile([128, NS], BF16, name="xstg", tag="xstg")
                nc.scalar.activation(stg[:], px[:], AF.Identity,
                                     bias=b_comb[:, g:g + 1], scale=SG)
                nc.sync.dma_start(xembT_d[g * 128:(g + 1) * 128, 0:NS], stg[:])

        trash_holder.clear()

        # ============ WEIGHTS (loop phase) ============
        ppsum = inner.enter_context(tc.tile_pool(name="ppsum", bufs=1,
                                                 space="PSUM"))
        Eps_A = ppsum.tile([128, TpA], F32)
        Eps_B = ppsum.tile([128, LB], F32)
        gall_tiles = [ppsum.tile([128, GC * BL], F32, name=f"gall{i}",
                                 tag=f"gall{i}") for i in range(2)]
        w_pool = inner.enter_context(tc.tile_pool(name="w", bufs=1))
        W_comb = [w_pool.tile([128, 4 * H], WDT, name=f"wc{k}", tag=f"wc{k}")
                  for k in range(DC + HC)]
        for k in range(DC + HC):
            nc.sync.dma_start(W_comb[k][:], W_combT_d[k * 128:(k + 1) * 128, :])
        W_sT = [w_pool.tile([128, A], BF16, name=f"ws{k}", tag=f"ws{k}")
                for k in range(HC)]
        for k in range(HC):
            nc.sync.dma_start(W_sT[k][:], W_sT_d[k * 128:(k + 1) * 128, :])

        qbcA = None  # set below (persistent, loop-carried)
        loop_sb = inner.enter_context(tc.tile_pool(name="lsb", bufs=2))
        loop_z = inner.enter_context(tc.tile_pool(name="lz", bufs=2))
        loop_q = inner.enter_context(tc.tile_pool(name="lq", bufs=2))
        loop_se = inner.enter_context(tc.tile_pool(name="lse", bufs=1))
        enc_pool = inner.enter_context(tc.tile_pool(name="encp", bufs=1))
        loop_ps2 = inner.enter_context(tc.tile_pool(name="lps2", bufs=1,
                                                    space="PSUM"))

        # enc resident in SBUF for the whole loop (identical every step)
        enc_all = []
        for j in range(BL):
            ea = enc_pool.tile([128, TC[j] * 512], BF16, name=f"ea{j}",
                               tag=f"ea{j}")
            nc.sync.dma_start(
                ea[:], enc_td[j][:, :].rearrange("(c p) d -> p c d", p=128))
            enc_all.append(ea)

        nc.vector.memset(Eps_A[:], 0.0)
        nc.vector.memset(Eps_B[:], 0.0)

        def emit_ghh(gall, g0, g1):
            # the gall tile is opened by a full-tile xet identity-matmul
            # (start=True); everything after accumulates onto written bytes.
            for g in range(g0, g1):
                for k in range(HC):
                    nc.tensor.matmul(gall[:, g * BL:(g + 1) * BL],
                                     W_comb[DC + k][:, g * 128:(g + 1) * 128],
                                     h_q[:, k * BL:(k + 1) * BL],
                                     start=False, stop=False,
                                     skip_group_check=True)

        # prologue: xemb(0) opens the psum tile, then W_hh part with h=0
        gall_cur = gall_tiles[0]
        xet0 = loop_sb.tile([128, GC * BL], BF16, name="xet", tag="xet")
        nc.sync.dma_start(xet0[:], xembT_d[:, 0:BL].rearrange(
            "(c p) b -> p c b", p=128))
        nc.tensor.matmul(gall_cur[:, 0:GC * BL], ident[:, :], xet0[:, :],
                         start=True, stop=False, skip_group_check=True)
        emit_ghh(gall_cur, 0, GC)
        # initial accum (=0) -> qd -> qbc (persistent loop-carried tiles)
        for j in range(BL):
            nc.sync.dma_start(qd[0:1, j * 1024:(j + 1) * 1024],
                              accum_bf[32 * j:32 * j + 1, :])
        qbcA = loop_q.tile([128, LA], BF16, name="qbcA", tag="qbcA")
        for ji, j in enumerate(GA):
            nc.sync.dma_start(
                qbcA[:, offA[ji]:offA[ji] + Tp[j]],
                qd[0:1, j * 1024:j * 1024 + Tp[j]].partition_broadcast(128))
        qbcB = loop_q.tile([128, LB], BF16, name="qbcB", tag="qbcB")
        nc.sync.dma_start(
            qbcB[:, 0:LB],
            qd[0:1, 3 * 1024:3 * 1024 + LB].partition_broadcast(128))

        # ============ STEP LOOP ============
        _nsteps = int(os.environ.get("KBSTEPS", S))
        UNROLL = int(os.environ.get("KBUNROLL", 4))

        last_sc = [None]
        scount = [0]

        def step_body(t4, par):
            gall = gall_tiles[par % 2]

            # ---- gates: ctx part (xemb + W_hh already accumulated) ----
            for g in range(GC):
                for k in range(DC):
                    nc.tensor.matmul(gall[:, g * BL:(g + 1) * BL],
                                     W_comb[k][:, g * 128:(g + 1) * 128],
                                     ctx_q[:, k * BL:(k + 1) * BL],
                                     start=False, stop=(g == GC - 1 and
                                                        k == DC - 1),
                                     skip_group_check=True)

            if dbg2_d is not None:
                sidx = scount[0]
                scount[0] += 1
                dtile = loop_sb.tile([128, GC * BL], F32, name="dtile",
                                     tag="dtile")
                nc.vector.tensor_copy(dtile[:], gall[:, :])
                nc.sync.dma_start(
                    dbg2_d[:, 64 + (sidx % 4) * 128:64 + (sidx % 4) * 128 + 128],
                    dtile[:])

            # ---- LSTM pointwise (fused) ----
            HB = HC * BL
            t_ifo = loop_sb.tile([128, 3 * HB], F32, name="t_ifo", tag="ti")
            nc.scalar.activation(t_ifo[:], gall[:, 0:3 * HB], AF.Tanh,
                                 scale=0.5 / SG)
            t_g = loop_sb.tile([128, HB], F32, name="t_g", tag="tg")
            nc.scalar.activation(t_g[:], gall[:, 3 * HB:], AF.Tanh,
                                 scale=1.0 / SG)
            av = loop_sb.tile([128, HB], F32, name="av", tag="av")
            nc.vector.scalar_tensor_tensor(av[:], t_ifo[:, HB:2 * HB], 1.0,
                                           c_st[:], ALU.add, ALU.mult)
            bv = loop_sb.tile([128, HB], F32, name="bv", tag="bv")
            nc.vector.scalar_tensor_tensor(bv[:], t_ifo[:, 0:HB], 1.0,
                                           t_g[:], ALU.add, ALU.mult)
            cn2 = loop_sb.tile([128, HB], F32, name="cn2", tag="cn2")
            nc.vector.tensor_tensor(cn2[:], av[:], bv[:], ALU.add)
            nc.vector.scalar_tensor_tensor(c_st[:], cn2[:], 0.5 * (1.0 - ZC),
                                           c05[:], ALU.mult, ALU.add)
            tcn = loop_sb.tile([128, HB], F32, name="tcn", tag="tcn")
            nc.scalar.activation(tcn[:], cn2[:], AF.Tanh, scale=0.5)
            hn2 = loop_sb.tile([128, HB], F32, name="hn2", tag="hn2")
            nc.vector.scalar_tensor_tensor(hn2[:], t_ifo[:, 2 * HB:3 * HB], 1.0,
                                           tcn[:], ALU.add, ALU.mult)
            nc.vector.scalar_tensor_tensor(h_bf[:], hn2[:], 0.5 * (1.0 - ZH),
                                           h05[:], ALU.mult, ALU.add)
            nc.vector.tensor_scalar(h_q[:], h_bf[:], SX, None, ALU.mult)
            dst = hstk_d[:, bass.ds(t4, BL)].rearrange("(c p) b -> p c b", p=128)
            nc.sync.dma_start(dst, h_bf[:])

            # ---- s' = h @ (W_s/wfb).T  (a-major so z can start early) ----
            # sc_ps holds s' in cols [0:32] and ctx in cols [32:48] (one
            # PSUM bank for both).
            COFF = AC * BL
            sc_ps = loop_ps2.tile([128, AC * BL + DC * BL], F32, name="sc_ps",
                                  tag="sc_ps")
            last_sc[0] = sc_ps
            nc.tensor.matmul(sc_ps[:, 0:COFF + DC * BL], zrow[0:1, 0:128],
                             zrow[0:1, 0:COFF + DC * BL],
                             start=True, stop=False, skip_group_check=True)
            for a in range(AC):
                for k in range(HC):
                    nc.tensor.matmul(sc_ps[:, a * BL:(a + 1) * BL],
                                     W_sT[k][:, a * 128:(a + 1) * 128],
                                     h_bf[:, k * BL:(k + 1) * BL],
                                     start=False,
                                     stop=(a == AC - 1 and k == HC - 1),
                                     skip_group_check=True)

            # ---- group A stretch ----
            for a in range(AC):
                z = loop_z.tile([128, LA], BF16, name="zA", tag="zA")
                for ji, j in enumerate(GA):
                    nc.vector.scalar_tensor_tensor(
                        z[:, offA[ji]:offA[ji] + Tp[j]],
                        e_A[a][:, offA[ji]:offA[ji] + Tp[j]],
                        sc_ps[:, a * BL + j:a * BL + j + 1],
                        qbcA[:, offA[ji]:offA[ji] + Tp[j]],
                        ALU.add, ALU.add)
                tv = loop_z.tile([128, LA], BF16, name="tvA", tag="tvA")
                nc.scalar.activation(tv[:], z[:], AF.Tanh,
                                     scale=wfb_colf[:, a:a + 1])
                for ji, j in enumerate(GA):
                    kw = dict(start=(a == 0), stop=(a == AC - 1),
                              skip_group_check=True)
                    if j:
                        kw["tile_position"] = (0, 32 * j)
                    for n0 in range(0, Tp[j], 512):
                        n1 = min(n0 + 512, Tp[j])
                        nc.tensor.matmul(Eps_A[32 * j:32 * j + 1, n0:n1],
                                         vT_col[:, a:a + 1],
                                         tv[:, offA[ji] + n0:offA[ji] + n1],
                                         **kw)
                # interleave: xemb prefetch + W_hh part of NEXT step's gates
                if a == 0:
                    gnext = gall_tiles[(par + 1) % 2]
                    xet = loop_sb.tile([128, GC * BL], BF16, name="xet",
                                       tag="xet")
                    nc.sync.dma_start(xet[:], xembT_d[:, bass.ds(t4 + BL, BL)]
                                      .rearrange("(c p) b -> p c b", p=128))
                    nc.tensor.matmul(gnext[:, 0:GC * BL], ident[:, :],
                                     xet[:, :], start=True, stop=False,
                                     skip_group_check=True)
                else:
                    gnext = gall_tiles[(par + 1) % 2]
                emit_ghh(gnext, a * (GC // AC), (a + 1) * (GC // AC))
                if a == 0:
                    # zoneout prefactors for the next step
                    nc.vector.tensor_scalar(c05[:], c_st[:], ZC, None, ALU.mult)
                    nc.vector.tensor_scalar(h05[:], h_bf[:], ZH, None, ALU.mult)

            # ---- group A tail (overlaps group B stretch) ----
            EnA = loop_se.tile([128, TpA], BF16, name="EnA", tag="EnA")
            nc.scalar.activation(EnA[:], Eps_A[:], AF.Exp)
            w1A = loop_se.tile([128, TpA], BF16, name="w1A", tag="w1A")
            seA = loop_se.tile([128, 1], F32, name="seA", tag="seA")
            nc.vector.scalar_tensor_tensor(w1A[0:96, :], EnA[0:96, :], 1.0,
                                           mask01[0:96, 0:TpA],
                                           ALU.mult, ALU.mult,
                                           accum_out=seA[0:96, :])
            rseA = loop_se.tile([128, 1], F32, name="rseA", tag="rseA")
            nc.vector.reciprocal(rseA[0:96, :], seA[0:96, :])
            nc.vector.tensor_scalar(w_att[0:96, 0:TpA], w1A[0:96, :],
                                    rseA[0:96, :], None, ALU.mult)
            nc.vector.scalar_tensor_tensor(EnA[0:96, :], w1A[0:96, :],
                                           rseA[0:96, :], finv[0:96, 0:TpA],
                                           ALU.mult, ALU.mult)
            nc.vector.tensor_tensor(accum_bf[0:96, 0:TpA], accum_bf[0:96, 0:TpA],
                                    EnA[0:96, :], ALU.add)
            for j in GA:
                nc.sync.dma_start(qd[0:1, j * 1024:(j + 1) * 1024],
                                  accum_bf[32 * j:32 * j + 1, :])
            for ji, j in enumerate(GA):
                nc.sync.dma_start(
                    qbcA[:, offA[ji]:offA[ji] + Tp[j]],
                    qd[0:1, j * 1024:j * 1024 + Tp[j]].partition_broadcast(128))

            # ---- group B stretch, interleaved with group A transposes and
            #      ctx matvecs ----
            for a in range(AC):
                zb = loop_z.tile([128, LB], BF16, name="zB", tag="zB")
                nc.vector.scalar_tensor_tensor(
                    zb[:, 0:LB], e_B[a][:, 0:LB],
                    sc_ps[:, a * BL + 3:a * BL + 4],
                    qbcB[:, 0:LB], ALU.add, ALU.add)
                tvb = loop_z.tile([128, LB], BF16, name="tvB", tag="tvB")
                nc.scalar.activation(tvb[:], zb[:], AF.Tanh,
                                     scale=wfb_colf[:, a:a + 1])
                kw = dict(start=(a == 0), stop=(a == AC - 1),
                          skip_group_check=True, tile_position=(0, 96))
                for n0 in range(0, LB, 512):
                    n1 = min(n0 + 512, LB)
                    nc.tensor.matmul(Eps_B[96:97, n0:n1], vT_col[:, a:a + 1],
                                     tvb[:, n0:n1], **kw)
                # group A transpose + ctx matvec for t-chunk c = a
                c = a
                if c < TC[0]:
                    wtp = loop_ps2.tile([128, 96], BF16, name="wtp", tag="wtp")
                    nc.tensor.transpose(wtp[:, 0:96],
                                        w_att[0:96, c * 128:(c + 1) * 128],
                                        ident[0:96, 0:96])
                    nsl = sum(1 for j in GA if TC[j] > c)
                    nc.vector.tensor_copy(wts[:, c * BL:c * BL + nsl],
                                          wtp[:, 0:32 * nsl:32])
                    for j in GA:
                        if c >= TC[j]:
                            continue
                        for dk in range(DC):
                            nc.tensor.matmul(
                                sc_ps[:, COFF + dk * BL + j:COFF + dk * BL + j + 1],
                                enc_all[j][:, c * 512 + dk * 128:
                                           c * 512 + (dk + 1) * 128],
                                wts[:, c * BL + j:c * BL + j + 1],
                                start=False, stop=(c == TC[j] - 1),
                                skip_group_check=True)

            # ---- group B tail (serial) ----
            EnB = loop_se.tile([128, LB], BF16, name="EnB", tag="EnB")
            nc.scalar.activation(EnB[:], Eps_B[:], AF.Exp)
            w1B = loop_se.tile([128, LB], BF16, name="w1B", tag="w1B")
            seB = loop_se.tile([128, 1], F32, name="seB", tag="seB")
            nc.vector.scalar_tensor_tensor(w1B[96:128, :], EnB[96:128, :], 1.0,
                                           mask01[96:128, 0:LB],
                                           ALU.mult, ALU.mult,
                                           accum_out=seB[96:128, :])
            rseB = loop_se.tile([128, 1], F32, name="rseB", tag="rseB")
            nc.vector.reciprocal(rseB[96:128, :], seB[96:128, :])
            nc.vector.tensor_scalar(w_att[96:128, 0:LB], w1B[96:128, :],
                                    rseB[96:128, :], None, ALU.mult)
            nc.vector.scalar_tensor_tensor(EnB[96:128, :], w1B[96:128, :],
                                           rseB[96:128, :], finv[96:128, 0:LB],
                                           ALU.mult, ALU.mult)
            nc.vector.tensor_tensor(accum_bf[96:128, 0:LB],
                                    accum_bf[96:128, 0:LB],
                                    EnB[96:128, :], ALU.add)
            nc.sync.dma_start(qd[0:1, 3 * 1024:4 * 1024],
                              accum_bf[96:97, :])
            nc.sync.dma_start(
                qbcB[:, 0:LB],
                qd[0:1, 3 * 1024:3 * 1024 + LB].partition_broadcast(128))

            for c in range(TC[3]):
                wtp = loop_ps2.tile([128, 96], BF16, name="wtp", tag="wtp")
                nc.tensor.transpose(wtp[:, 0:32],
                                    w_att[96:128, c * 128:(c + 1) * 128],
                                    ident[96:128, 96:128],
                                    tile_position=(96, 0))
                nc.vector.tensor_copy(wts[:, c * BL + 3:c * BL + 4],
                                      wtp[:, 0:1])
                for dk in range(DC):
                    nc.tensor.matmul(
                        sc_ps[:, COFF + dk * BL + 3:COFF + dk * BL + 4],
                        enc_all[3][:, c * 512 + dk * 128:c * 512 + (dk + 1) * 128],
                        wts[:, c * BL + 3:c * BL + 4],
                        start=False, stop=(c == TC[3] - 1),
                        skip_group_check=True)

            nc.vector.tensor_copy(ctxT_sb[:], sc_ps[:, COFF:COFF + DC * BL])
            nc.vector.tensor_scalar(ctx_q[:], sc_ps[:, COFF:COFF + DC * BL], SX, None, ALU.mult)
            dst = cstk_d[:, bass.ds(t4, BL)].rearrange("(c p) b -> p c b", p=128)
            nc.sync.dma_start(dst, ctxT_sb[:])

        assert UNROLL % 2 == 0, "gall parity needs even UNROLL"
        with tc.For_i(0, _nsteps * BL, UNROLL * BL,
                      hint_engines=(ET.PE, ET.Activation, ET.DVE, ET.SP)) as t4:
            for s in range(UNROLL):
                step_body(t4 + s * BL, s)

        if dbg_d is not None:
            nc.sync.dma_start(dbg_d[:, 0:1024], w_att[:, :])
            nc.sync.dma_start(dbg_d[:, 1024:1024 + HC * BL], h_bf[:, :])
            nc.sync.dma_start(dbg_d[:, 1056:1056 + TCmax * BL], wts[:, :])
            pass

        # ============ READOUT ============
        inner.close()
        post_sb = ctx.enter_context(tc.tile_pool(name="post_sb", bufs=1))
        post_st = ctx.enter_context(tc.tile_pool(name="post_st", bufs=2))
        post_ps = ctx.enter_context(tc.tile_pool(name="post_ps", bufs=2,
                                                 space="PSUM"))
        trash_holder.append(post_ps.tile([128, 128], BF16, name="trash_post"))

        xro = []
        for k in range(HC):
            tl = post_sb.tile([128, NS], BF16, name=f"xh{k}", tag=f"xh{k}")
            nc.sync.dma_start(tl[:], hstk_d[k * 128:(k + 1) * 128, :])
            xro.append(tl)
        for k in range(EC):
            tl = post_sb.tile([128, NS], BF16, name=f"xe{k}", tag=f"xe{k}")
            nc.sync.dma_start(tl[:], embT_d[k * 128:(k + 1) * 128, :])
            xro.append(tl)
        for k in range(DC):
            tl = post_sb.tile([128, NS], BF16, name=f"xc{k}", tag=f"xc{k}")
            nc.sync.dma_start(tl[:], cstk_d[k * 128:(k + 1) * 128, :])
            xro.append(tl)
        W_roe = [post_sb.tile([128, RO // 2], BF16, name=f"wre{k}", tag=f"wre{k}")
                 for k in range(XROC)]
        W_roo = [post_sb.tile([128, RO // 2], BF16, name=f"wro{k}", tag=f"wro{k}")
                 for k in range(XROC)]
        for k in range(XROC):
            nc.sync.dma_start(W_roe[k][:], W_roT_e_d[k * 128:(k + 1) * 128, :])
            nc.sync.dma_start(W_roo[k][:], W_roT_o_d[k * 128:(k + 1) * 128, :])
        b_ro_e = post_sb.tile([128, ROC], F32)
        nc.sync.dma_start(b_ro_e[:], b_ro_e_d[:, :])
        b_ro_o = post_sb.tile([128, ROC], F32)
        nc.sync.dma_start(b_ro_o[:], b_ro_o_d[:, :])
        b_out_col = post_sb.tile([128, VC], F32)
        nc.sync.dma_start(b_out_col[:], b_out_d[:, :])
        pe_touch(xro[0][:, 0:128])
        pe_touch(W_roe[0][:, 0:128])
        pe_touch(W_roo[0][:, 0:128])

        maxo = []
        for oc in range(ROC):
            Re = post_ps.tile([128, NS], F32, name="Re", tag="Re")
            for k in range(XROC):
                nc.tensor.matmul(Re[:], W_roe[k][:, oc * 128:(oc + 1) * 128],
                                 xro[k][:], start=(k == 0), stop=(k == XROC - 1))
            t1 = post_st.tile([128, NS], F32, name="t1", tag="t1")
            nc.scalar.activation(t1[:], Re[:], AF.Identity,
                                 bias=b_ro_e[:, oc:oc + 1], scale=1.0)
            Ro = post_ps.tile([128, NS], F32, name="Ro", tag="Re")
            for k in range(XROC):
                nc.tensor.matmul(Ro[:], W_roo[k][:, oc * 128:(oc + 1) * 128],
                                 xro[k][:], start=(k == 0), stop=(k == XROC - 1))
            t2 = post_st.tile([128, NS], F32, name="t2", tag="t2")
            nc.scalar.activation(t2[:], Ro[:], AF.Identity,
                                 bias=b_ro_o[:, oc:oc + 1], scale=1.0)
            mo = post_sb.tile([128, NS], BF16, name=f"mo{oc}", tag=f"mo{oc}")
            nc.vector.tensor_tensor(mo[:], t1[:], t2[:], ALU.max)
            maxo.append(mo)

        wo_pool = ctx.enter_context(tc.tile_pool(name="wo", bufs=6))
        first = True
        for vc in range(VC):
            wo = [wo_pool.tile([128, 128], BF16, name=f"wo{vc}_{k}", tag=f"wok{k}")
                  for k in range(ROC)]
            for k in range(ROC):
                nc.sync.dma_start(wo[k][:],
                                  W_outT_d[k * 128:(k + 1) * 128,
                                           vc * 128:(vc + 1) * 128])
            if first:
                pe_touch(wo[0][:, 0:128])
                pe_touch(maxo[0][:, 0:128])
                first = False
            L = post_ps.tile([128, NS], F32, name="L", tag="L")
            for k in range(ROC):
                nc.tensor.matmul(L[:], wo[k][:], maxo[k][:],
                                 start=(k == 0), stop=(k == ROC - 1))
            lo = post_st.tile([128, NS], F32, name="lo", tag="lo")
            nc.scalar.activation(lo[:], L[:], AF.Identity,
                                 bias=b_out_col[:, vc:vc + 1], scale=1.0)
            nc.sync.dma_start(out_d[vc * 128:(vc + 1) * 128, :], lo[:])

    return nc


def check_waits(nc, cap_note=""):
    bad = []
    for fn in nc.m.functions:
        for bb in fn.blocks:
            for inst in bb.instructions:
                c = inst.concise()
                nw = c.count("wait:")
                eng = c.split()[0] if c.split() else "?"
                if nw >= 2 and eng in ("PE", "ACT", "DVE", "PL"):
                    bad.append((nw, c[:180]))
    for nw, c in bad:
        print("WAITS", nw, c)
    return bad


def _prep_core(inputs, order, Tp, core):
    enc = np.asarray(inputs["encoder_outputs"], np.float32)
    labels = np.asarray(inputs["labels"])
    lens = np.asarray(inputs["enc_seq_len"], np.int64)
    embed = np.asarray(inputs["embed"], np.float32)

    bidx = [int(order[j * NCORE + core]) for j in range(BL)]
    m = {}
    for j in range(BL):
        b = bidx[j]
        ep = np.zeros((Tp[j], D), np.float32)
        ep[:T] = enc[b, :Tp[j] if Tp[j] <= T else T]
        m[f"enc_td{j}"] = _bf(ep)
        m[f"encT{j}"] = _bf(ep.T)
    emb = np.zeros((BL, S, E), np.float32)
    for j in range(BL):
        b = bidx[j]
        emb[j, 1:] = embed[labels[b, :S - 1].astype(np.int64)]
    embT = emb.transpose(2, 1, 0).reshape(E, NS)
    m["embT"] = _bf(embT)
    mask01 = np.zeros((BL, 1024), np.float32)
    for j in range(BL):
        mask01[j, :int(lens[bidx[j]])] = 1.0
    m["mask01"] = _bf(mask01)
    return m, bidx


def kernel(**inputs):
    lens = np.asarray(inputs["enc_seq_len"], np.int64)
    order = np.argsort(-lens, kind="stable")
    Tp = []
    for j in range(BL):
        mx = max(int(lens[order[j * NCORE + i]]) for i in range(NCORE))
        Tp.append(min(1024, ((mx + 127) // 128) * 128))

    perm = _gate_perm()
    W_ih = np.asarray(inputs["W_ih"], np.float32)[perm]
    W_hh = np.asarray(inputs["W_hh"], np.float32)[perm]
    b_sum = (np.asarray(inputs["b_ih"], np.float32)
             + np.asarray(inputs["b_hh"], np.float32))[perm]
    wfb = np.asarray(inputs["W_fb"], np.float32)[:, 0]
    wfb_safe = np.where(wfb >= 0, np.maximum(wfb, 1e-3),
                        np.minimum(wfb, -1e-3))
    shared = {
        "W_combT": _w8(np.concatenate([W_ih[:, E:].T, W_hh.T], 0)),
        "W_ih_embT": _bf(W_ih[:, :E].T),
        "W_encT": _bf(np.asarray(inputs["W_enc"], np.float32).T
                      / wfb_safe[None, :]),
        "W_sT": _bf(np.asarray(inputs["W_s"], np.float32).T
                    / wfb_safe[None, :]),
        "wfert_col": _bf(np.asarray(inputs["W_fert"],
                                    np.float32).reshape(DC, 128).T),
        "vT_col": _bf(np.asarray(inputs["v_att"], np.float32).reshape(AC, 128).T),
        "wfb_colf": np.ascontiguousarray(
            wfb_safe.reshape(AC, 128).T.astype(np.float32)),
        "b_enc_col": np.ascontiguousarray(
            (np.asarray(inputs["b_enc"], np.float32) / wfb_safe)
            .reshape(AC, 128).T),
        "b_comb": np.ascontiguousarray(
            (b_sum * SG).reshape(GC, 128).T),
        "W_roT_e": _bf(np.asarray(inputs["W_ro"], np.float32)[0::2].T),
        "W_roT_o": _bf(np.asarray(inputs["W_ro"], np.float32)[1::2].T),
        "b_ro_e": np.ascontiguousarray(
            np.asarray(inputs["b_ro"], np.float32)[0::2].reshape(ROC, 128).T),
        "b_ro_o": np.ascontiguousarray(
            np.asarray(inputs["b_ro"], np.float32)[1::2].reshape(ROC, 128).T),
        "W_outT": _bf(np.asarray(inputs["W_out"], np.float32).T),
        "b_out_col": np.ascontiguousarray(
            np.asarray(inputs["b_out"], np.float32).reshape(VC, 128).T),
    }

    in_maps = []
    bidx_all = []
    for c in range(NCORE):
        m, bidx = _prep_core(inputs, order, Tp, c)
        m.update(shared)
        in_maps.append(m)
        bidx_all.append(bidx)

    nc = build_nc(Tp)
    nc.finalize()
    from concourse.bass_utils import run_bass_kernel_spmd
    trace = bool(os.environ.get("BASS_KERNEL_TRACE"))
    res = run_bass_kernel_spmd(nc, in_maps, core_ids=list(range(NCORE)),
                               trace=trace)
    global LAST_EXEC_NS, LAST_OUTS, LAST_META
    LAST_EXEC_NS = res.exec_time_ns
    outs = res.results
    LAST_OUTS = outs
    LAST_META = (order, Tp, bidx_all)

    logits = np.zeros((B, S, V), np.float32)
    for c in range(NCORE):
        o = outs[c]["out"].reshape(V, S, BL)
        for j in range(BL):
            logits[bidx_all[c][j]] = o[:, :, j].T
    return logits


if __name__ == "__main__":
    nc = build_nc([1024, 896, 768, 640])
    bad = check_waits(nc)
    print(f"{len(bad)} instructions with >=2 waits")


# revision 34
# speedup vs baseline: 1.2390x; 1.2390x over previous
"""Attention-LSTM decoder (B=32, T=1000, S=100, D=512, A=1024, H=1024,
E=640, V=10240, P=1024) on 8 trn2 NeuronCores.

Sharding: data-parallel over batch, 4 batches per core (one per "slot").
Batches are sorted by enc_seq_len; slot j holds ranks [j*8:(j+1)*8] so the
padded time extent Tp[j] (multiple of 128) is shared by all 8 cores and the
SPMD graph is identical across cores.

v3 design (vs. v2 baseline):
  - attention slots split into group A = slots {0,1,2} and group B = {3}.
    Group A's softmax / transpose / ctx-matvec tail executes underneath
    group B's tanh stretch; only B's short tail is serial.
  - tanh merged per (group, a-chunk): s_t/wfb is folded into the z-add via
    scalar_tensor_tensor with the per-partition scalar read directly from
    the s PSUM tile (host pre-divides W_s rows by wfb), so one ACT
    instruction covers all slots of a group.
  - softmax: exp -> one STT that applies the {0,1} mask AND emits the row
    sums via accum_out (no tensor_reduce); w*finv fused the same way.
  - gates accumulate in a single PSUM tile: W_hh part prefetched during the
    previous stretch, ctx part + xemb (via identity matmul) appended, and
    the activations read PSUM directly.
  - LSTM pointwise lowered to 5 STT + 1 TT + 1 TS using 2*sigmoid(x) =
    tanh(x/2) + 1; zoneout blends use pre-scaled c05/h05 computed during
    the previous stretch.
"""
import sys

sys.path.insert(0, "/opt/trn_rl_repo")

import os
import numpy as np
import ml_dtypes
from contextlib import ExitStack

import concourse.bass as bass
import concourse.tile as tile
import concourse.mybir as mybir
from concourse import bacc
from concourse.masks import make_identity

DT = mybir.dt
F32 = DT.float32
BF16 = DT.bfloat16
FP8 = DT.float8e4
AF = mybir.ActivationFunctionType
ALU = mybir.AluOpType
ET = mybir.EngineType

B, T, S = 32, 1000, 100
D, A, H, E, V, RO = 512, 1024, 1024, 640, 10240, 1024
ZH, ZC = 0.05, 0.15
NCORE = 8
BL = B // NCORE          # 4 batches (slots) per core
NS = S * BL              # 400 step-batch columns
GC = 4 * H // 128        # 32 gate chunks
HC = H // 128            # 8
AC = A // 128            # 8
DC = D // 128            # 4
EC = E // 128            # 5
ROC = RO // 2 // 128     # 4 chunks per maxout half
VC = V // 128            # 80 vocab chunks
XROC = (H + E + D) // 128  # 17 readout K-chunks

USE_FP8 = os.environ.get("KBFP8", "1") != "0"
SW = 64.0 if USE_FP8 else 1.0    # weight scale
SX = 16.0 if USE_FP8 else 1.0    # moving (h/ctx) scale
SG = SW * SX                      # psum scale for gates
WDT = FP8 if USE_FP8 else BF16

GA = (0, 1, 2)  # group A slots
GB = (3,)       # group B slots

bf16 = ml_dtypes.bfloat16
f8 = ml_dtypes.float8_e4m3
LAST_EXEC_NS = None
LAST_OUTS = None
LAST_META = None


def _bf(a):
    return np.ascontiguousarray(np.asarray(a, dtype=np.float32)).astype(bf16)


def _w8(a):
    a = np.asarray(a, dtype=np.float32) * SW
    return np.ascontiguousarray(a).astype(f8 if USE_FP8 else bf16)


# gate-permutation: reference gate order is [i|f|g|o]; we reorder rows to
# [i|f|o|g] so the three sigmoids are contiguous.
def _gate_perm():
    idx = np.arange(4 * H)
    return np.concatenate([idx[0:2 * H], idx[3 * H:4 * H], idx[2 * H:3 * H]])


def build_nc(Tp, debug=False):
    TC = [t // 128 for t in Tp]
    TCmax = max(TC)
    offA = [0, Tp[0], Tp[0] + Tp[1]]       # segment offsets in group-A tiles
    LA = Tp[0] + Tp[1] + Tp[2]
    LB = Tp[3]
    TpA = Tp[0]                            # group-A col extent (max of group)
    nc = bacc.Bacc("TRN2", target_bir_lowering=False)

    def param(name, shape, dt=BF16):
        return nc.declare_dram_parameter(name, list(shape), dt, isOutput=False)

    enc_td = [param(f"enc_td{j}", [Tp[j], D]) for j in range(BL)]
    encT = [param(f"encT{j}", [D, Tp[j]]) for j in range(BL)]
    embT_d = param("embT", [E, NS])
    W_combT_d = param("W_combT", [D + H, 4 * H], WDT)
    W_ih_embT_d = param("W_ih_embT", [E, 4 * H])
    W_encT_d = param("W_encT", [D, A])
    W_sT_d = param("W_sT", [H, A])                 # bf16: W_s / wfb
    wfert_col_d = param("wfert_col", [128, DC])
    vT_col_d = param("vT_col", [128, AC])
    wfb_colf_d = param("wfb_colf", [128, AC], F32)
    b_enc_col_d = param("b_enc_col", [128, AC], F32)
    b_comb_d = param("b_comb", [128, GC], F32)     # pre-scaled by SG on host
    mask01_d = param("mask01", [BL, 1024])         # {0,1} rows
    W_roT_e_d = param("W_roT_e", [H + E + D, RO // 2])
    W_roT_o_d = param("W_roT_o", [H + E + D, RO // 2])
    b_ro_e_d = param("b_ro_e", [128, ROC], F32)
    b_ro_o_d = param("b_ro_o", [128, ROC], F32)
    W_outT_d = param("W_outT", [RO // 2, V])
    b_out_d = param("b_out_col", [128, VC], F32)
    out_d = nc.declare_dram_parameter("out", [V, NS], F32, isOutput=True)

    qd = nc.dram_tensor("qd", [1, BL * 1024], BF16)
    dbg_d = nc.declare_dram_parameter("dbgt", [128, 1088], BF16,
                                      isOutput=True) \
        if os.environ.get("KBDBG") else None
    dbg2_d = nc.declare_dram_parameter("dbgt2", [128, 576], F32,
                                       isOutput=True) \
        if os.environ.get("KBDBG") else None
    hstk_d = nc.dram_tensor("hstk", [H, NS], BF16)
    cstk_d = nc.dram_tensor("cstk", [D, NS], BF16)
    xembT_d = nc.dram_tensor("xembT", [4 * H, NS + BL], BF16)

    with ExitStack() as ctx:
        tc = ctx.enter_context(tile.TileContext(nc))

        # ---------------- persistent pools ----------------
        persist = ctx.enter_context(tc.tile_pool(name="persist", bufs=1))
        ident = persist.tile([128, 128], BF16)
        make_identity(nc, ident[:])
        vT_col = persist.tile([128, AC], BF16)
        nc.sync.dma_start(vT_col[:], vT_col_d[:, :])
        wfb_colf = persist.tile([128, AC], F32)
        nc.sync.dma_start(wfb_colf[:], wfb_colf_d[:, :])
        wfert_col = persist.tile([128, DC], BF16)
        nc.sync.dma_start(wfert_col[:], wfert_col_d[:, :])
        b_enc_col = persist.tile([128, AC], F32)
        nc.sync.dma_start(b_enc_col[:], b_enc_col_d[:, :])
        b_comb = persist.tile([128, GC], F32)
        nc.sync.dma_start(b_comb[:], b_comb_d[:, :])
        mask01 = persist.tile([128, 1024], BF16)
        nc.vector.memset(mask01[:], 0.0)
        # col 0 = 1 on every row so dead-row softmax sums stay finite
        nc.vector.memset(mask01[:, 0:1], 1.0)
        for j in range(BL):
            nc.sync.dma_start(mask01[32 * j:32 * j + 1, :], mask01_d[j:j + 1, :])

        h_bf = persist.tile([128, HC * BL], BF16)
        h_q = persist.tile([128, HC * BL], WDT)
        c_st = persist.tile([128, HC * BL], F32)
        c05 = persist.tile([128, HC * BL], F32)
        h05 = persist.tile([128, HC * BL], F32)
        ctxT_sb = persist.tile([128, DC * BL], BF16)
        ctx_q = persist.tile([128, DC * BL], WDT)
        accum_bf = persist.tile([128, 1024], BF16)  # rows {0,32,64,96}
        w_att = persist.tile([128, 1024], BF16)
        finv = persist.tile([128, 1024], BF16)     # rows {0,32,64,96}, x0.5
        wts = persist.tile([128, TCmax * BL], BF16)
        s_sb = persist.tile([128, AC * BL], F32)
        zrow = persist.tile([1, 128], BF16)
        nc.vector.memset(zrow[:], 0.0)
        zpad = persist.tile([128, GC * BL], BF16)
        nc.vector.memset(zpad[:], 0.0)
        nc.sync.dma_start(
            xembT_d[:, NS:NS + BL].rearrange("(c p) b -> p c b", p=128),
            zpad[:])
        for t_ in (h_bf, h_q, c_st, c05, h05, ctxT_sb, ctx_q, accum_bf,
                   w_att, finv, wts):
            nc.vector.memset(t_[:], 0.0)

        inner = ctx.enter_context(ExitStack())
        e_pool = inner.enter_context(tc.tile_pool(name="e", bufs=1))
        e_A = [e_pool.tile([128, LA], BF16, name=f"eA{a}", tag=f"eA{a}")
               for a in range(AC)]
        e_B = [e_pool.tile([128, LB], BF16, name=f"eB{a}", tag=f"eB{a}")
               for a in range(AC)]

        trash_holder = []

        def pe_touch(ap):
            # phase-scoped trash tile (pre/post only; fp8 touches are no-ops)
            if ap.dtype not in (BF16,) or not trash_holder:
                return
            trash_ps = trash_holder[0]
            p = ap.shape[0]
            nc.tensor.transpose(trash_ps[0:min(ap.shape[1], 128), 0:p],
                                ap[:, 0:min(ap.shape[1], 128)], ident[0:p, 0:p])

        # ============ PRECOMPUTE PHASE ============
        with ExitStack() as pre:
            pre_sb = pre.enter_context(tc.tile_pool(name="pre_sb", bufs=1))
            pre_st = pre.enter_context(tc.tile_pool(name="pre_st", bufs=2))
            pre_ps = pre.enter_context(tc.tile_pool(name="pre_ps", bufs=1,
                                                    space="PSUM"))
            trash_holder.append(pre_ps.tile([128, 128], BF16, name="trash_pre"))

            W_encT = [pre_sb.tile([128, A], BF16, name=f"wenc{k}", tag=f"we{k}")
                      for k in range(DC)]
            for k in range(DC):
                nc.sync.dma_start(W_encT[k][:], W_encT_d[k * 128:(k + 1) * 128, :])
            pe_touch(W_encT[0][:, 0:128])

            for j in range(BL):
                ercs = [pre_st.tile([128, Tp[j]], BF16, name=f"erc{j}{k}",
                                    tag=f"erc{k}") for k in range(DC)]
                for k in range(DC):
                    nc.sync.dma_start(ercs[k][:], encT[j][k * 128:(k + 1) * 128, :])
                    pe_touch(ercs[k][:, 0:128])
                for a in range(AC):
                    pe2 = pre_ps.tile([128, 1024], F32, name="pe_e2", tag="pe_e2")
                    for k in range(DC):
                        for n0 in range(0, Tp[j], 512):
                            n1 = min(n0 + 512, Tp[j])
                            nc.tensor.matmul(pe2[:, n0:n1],
                                             W_encT[k][:, a * 128:(a + 1) * 128],
                                             ercs[k][:, n0:n1],
                                             start=(k == 0), stop=(k == DC - 1))
                    if j in GA:
                        dst = e_A[a][:, offA[j]:offA[j] + Tp[j]]
                    else:
                        dst = e_B[a][:, 0:Tp[j]]
                    nc.scalar.activation(dst, pe2[:, 0:Tp[j]],
                                         AF.Identity,
                                         bias=b_enc_col[:, a:a + 1], scale=1.0)
                pf = pre_ps.tile([1, 1024], F32, name="pf", tag="pf")
                for k in range(DC):
                    for n0 in range(0, Tp[j], 512):
                        n1 = min(n0 + 512, Tp[j])
                        nc.tensor.matmul(pf[0:1, n0:n1], wfert_col[:, k:k + 1],
                                         ercs[k][:, n0:n1],
                                         start=(k == 0), stop=(k == DC - 1))
                # finv = 0.5*sigmoid(x) = 0.25*tanh(0.5x) + 0.25  (no table sw)
                fstage = pre_st.tile([1, 1024], F32, name="fstage", tag="fstage")
                nc.scalar.activation(fstage[0:1, 0:Tp[j]], pf[0:1, 0:Tp[j]],
                                     AF.Tanh, scale=0.5)
                fst2 = pre_st.tile([1, 1024], BF16, name="fst2", tag="fst2")
                nc.vector.tensor_scalar(fst2[0:1, 0:Tp[j]], fstage[0:1, 0:Tp[j]],
                                        0.25, 0.25, ALU.mult, ALU.add)
                nc.sync.dma_start(finv[32 * j:32 * j + 1, 0:Tp[j]],
                                  fst2[0:1, 0:Tp[j]])

            embT_sb = [pre_sb.tile([128, NS], BF16, name=f"embs{k}", tag=f"em{k}")
                       for k in range(EC)]
            for k in range(EC):
                nc.sync.dma_start(embT_sb[k][:], embT_d[k * 128:(k + 1) * 128, :])
            W_ie = [pre_sb.tile([128, 4 * H], BF16, name=f"wie{k}", tag=f"wi{k}")
                    for k in range(EC)]
            for k in range(EC):
                nc.sync.dma_start(W_ie[k][:], W_ih_embT_d[k * 128:(k + 1) * 128, :])
            pe_touch(W_ie[0][:, 0:128])
            pe_touch(embT_sb[0][:, 0:128])
            for g in range(GC):
                px = pre_ps.tile([128, NS], F32, name="px", tag="pe_e2")
                for k in range(EC):
                    nc.tensor.matmul(px[:], W_ie[k][:, g * 128:(g + 1) * 128],
                                     embT_sb[k][:], start=(k == 0),
                                     stop=(k == EC - 1))
                # xemb scaled by SG, bias pre-scaled on host
                stg = pre_st.tile([128, NS], BF16, name="xstg", tag="xstg")
                nc.scalar.activation(stg[:], px[:], AF.Identity,
                                     bias=b_comb[:, g:g + 1], scale=SG)
                nc.sync.dma_start(xembT_d[g * 128:(g + 1) * 128, 0:NS], stg[:])

        trash_holder.clear()

        # ============ WEIGHTS (loop phase) ============
        ppsum = inner.enter_context(tc.tile_pool(name="ppsum", bufs=1,
                                                 space="PSUM"))
        Eps_A = ppsum.tile([128, TpA], F32)
        Eps_B = ppsum.tile([128, LB], F32)
        gall_tiles = [ppsum.tile([128, GC * BL], F32, name=f"gall{i}",
                                 tag=f"gall{i}") for i in range(2)]
        w_pool = inner.enter_context(tc.tile_pool(name="w", bufs=1))
        W_comb = [w_pool.tile([128, 4 * H], WDT, name=f"wc{k}", tag=f"wc{k}")
                  for k in range(DC + HC)]
        for k in range(DC + HC):
            nc.sync.dma_start(W_comb[k][:], W_combT_d[k * 128:(k + 1) * 128, :])
        W_sT = [w_pool.tile([128, A], BF16, name=f"ws{k}", tag=f"ws{k}")
                for k in range(HC)]
        for k in range(HC):
            nc.sync.dma_start(W_sT[k][:], W_sT_d[k * 128:(k + 1) * 128, :])

        qbcA = None  # set below (persistent, loop-carried)
        loop_sb = inner.enter_context(tc.tile_pool(name="lsb", bufs=2))
        loop_z = inner.enter_context(tc.tile_pool(name="lz", bufs=2))
        loop_q = inner.enter_context(tc.tile_pool(name="lq", bufs=2))
        loop_se = inner.enter_context(tc.tile_pool(name="lse", bufs=1))
        enc_pool = inner.enter_context(tc.tile_pool(name="encp", bufs=1))
        loop_ps2 = inner.enter_context(tc.tile_pool(name="lps2", bufs=1,
                                                    space="PSUM"))

        # enc resident in SBUF for the whole loop (identical every step)
        enc_all = []
        for j in range(BL):
            ea = enc_pool.tile([128, TC[j] * 512], BF16, name=f"ea{j}",
                               tag=f"ea{j}")
            nc.sync.dma_start(
                ea[:], enc_td[j][:, :].rearrange("(c p) d -> p c d", p=128))
            enc_all.append(ea)

        nc.vector.memset(Eps_A[:], 0.0)
        nc.vector.memset(Eps_B[:], 0.0)

        def emit_ghh(gall, g0, g1):
            # the gall tile is opened by a full-tile xet identity-matmul
            # (start=True); everything after accumulates onto written bytes.
            for g in range(g0, g1):
                for k in range(HC):
                    nc.tensor.matmul(gall[:, g * BL:(g + 1) * BL],
                                     W_comb[DC + k][:, g * 128:(g + 1) * 128],
                                     h_q[:, k * BL:(k + 1) * BL],
                                     start=False, stop=False,
                                     skip_group_check=True)

        # prologue: xemb(0) opens the psum tile, then W_hh part with h=0
        gall_cur = gall_tiles[0]
        xet0 = loop_sb.tile([128, GC * BL], BF16, name="xet", tag="xet")
        nc.sync.dma_start(xet0[:], xembT_d[:, 0:BL].rearrange(
            "(c p) b -> p c b", p=128))
        nc.tensor.matmul(gall_cur[:, 0:GC * BL], ident[:, :], xet0[:, :],
                         start=True, stop=False, skip_group_check=True)
        emit_ghh(gall_cur, 0, GC)
        # initial accum (=0) -> qd -> qbc (persistent loop-carried tiles)
        for j in range(BL):
            nc.sync.dma_start(qd[0:1, j * 1024:(j + 1) * 1024],
                              accum_bf[32 * j:32 * j + 1, :])
        qbcA = loop_q.tile([128, LA], BF16, name="qbcA", tag="qbcA")
        for ji, j in enumerate(GA):
            nc.sync.dma_start(
                qbcA[:, offA[ji]:offA[ji] + Tp[j]],
                qd[0:1, j * 1024:j * 1024 + Tp[j]].partition_broadcast(128))
        qbcB = loop_q.tile([128, LB], BF16, name="qbcB", tag="qbcB")
        nc.sync.dma_start(
            qbcB[:, 0:LB],
            qd[0:1, 3 * 1024:3 * 1024 + LB].partition_broadcast(128))

        # ============ STEP LOOP ============
        _nsteps = int(os.environ.get("KBSTEPS", S))
        UNROLL = int(os.environ.get("KBUNROLL", 4))

        last_sc = [None]
        scount = [0]

        def step_body(t4, par):
            gall = gall_tiles[par % 2]

            # ---- gates: ctx part (xemb + W_hh already accumulated) ----
            for g in range(GC):
                for k in range(DC):
                    nc.tensor.matmul(gall[:, g * BL:(g + 1) * BL],
                                     W_comb[k][:, g * 128:(g + 1) * 128],
                                     ctx_q[:, k * BL:(k + 1) * BL],
                                     start=False, stop=(g == GC - 1 and
                                                        k == DC - 1),
                                     skip_group_check=True)

            if dbg2_d is not None:
                sidx = scount[0]
                scount[0] += 1
                dtile = loop_sb.tile([128, GC * BL], F32, name="dtile",
                                     tag="dtile")
                nc.vector.tensor_copy(dtile[:], gall[:, :])
                nc.sync.dma_start(
                    dbg2_d[:, 64 + (sidx % 4) * 128:64 + (sidx % 4) * 128 + 128],
                    dtile[:])

            # ---- LSTM pointwise (fused) ----
            HB = HC * BL
            t_ifo = loop_sb.tile([128, 3 * HB], F32, name="t_ifo", tag="ti")
            nc.scalar.activation(t_ifo[:], gall[:, 0:3 * HB], AF.Tanh,
                                 scale=0.5 / SG)
            t_g = loop_sb.tile([128, HB], F32, name="t_g", tag="tg")
            nc.scalar.activation(t_g[:], gall[:, 3 * HB:], AF.Tanh,
                                 scale=1.0 / SG)
            av = loop_sb.tile([128, HB], F32, name="av", tag="av")
            nc.vector.scalar_tensor_tensor(av[:], t_ifo[:, HB:2 * HB], 1.0,
                                           c_st[:], ALU.add, ALU.mult)
            bv = loop_sb.tile([128, HB], F32, name="bv", tag="bv")
            nc.vector.scalar_tensor_tensor(bv[:], t_ifo[:, 0:HB], 1.0,
                                           t_g[:], ALU.add, ALU.mult)
            cn2 = loop_sb.tile([128, HB], F32, name="cn2", tag="cn2")
            nc.vector.tensor_tensor(cn2[:], av[:], bv[:], ALU.add)
            nc.vector.scalar_tensor_tensor(c_st[:], cn2[:], 0.5 * (1.0 - ZC),
                                           c05[:], ALU.mult, ALU.add)
            tcn = loop_sb.tile([128, HB], F32, name="tcn", tag="tcn")
            nc.scalar.activation(tcn[:], cn2[:], AF.Tanh, scale=0.5)
            hn2 = loop_sb.tile([128, HB], F32, name="hn2", tag="hn2")
            nc.vector.scalar_tensor_tensor(hn2[:], t_ifo[:, 2 * HB:3 * HB], 1.0,
                                           tcn[:], ALU.add, ALU.mult)
            nc.vector.scalar_tensor_tensor(h_bf[:], hn2[:], 0.5 * (1.0 - ZH),
                                           h05[:], ALU.mult, ALU.add)
            nc.vector.tensor_scalar(h_q[:], h_bf[:], SX, None, ALU.mult)
            dst = hstk_d[:, bass.ds(t4, BL)].rearrange("(c p) b -> p c b", p=128)
            nc.sync.dma_start(dst, h_bf[:])

            # ---- s' = h @ (W_s/wfb).T  (a-major so z can start early) ----
            # sc_ps holds s' in cols [0:32] and ctx in cols [32:48] (one
            # PSUM bank for both).
            COFF = AC * BL
            sc_ps = loop_ps2.tile([128, AC * BL + DC * BL], F32, name="sc_ps",
                                  tag="sc_ps")
            last_sc[0] = sc_ps
            nc.tensor.matmul(sc_ps[:, 0:COFF + DC * BL], zrow[0:1, 0:128],
                             zrow[0:1, 0:COFF + DC * BL],
                             start=True, stop=False, skip_group_check=True)
            for a in range(AC):
                for k in range(HC):
                    nc.tensor.matmul(sc_ps[:, a * BL:(a + 1) * BL],
                                     W_sT[k][:, a * 128:(a + 1) * 128],
                                     h_bf[:, k * BL:(k + 1) * BL],
                                     start=False,
                                     stop=(a == AC - 1 and k == HC - 1),
                                     skip_group_check=True)
                nc.vector.tensor_copy(s_sb[:, a * BL:(a + 1) * BL],
                                      sc_ps[:, a * BL:(a + 1) * BL])

            # ---- group A stretch ----
            for a in range(AC):
                z = loop_z.tile([128, LA], BF16, name="zA", tag="zA")
                for ji, j in enumerate(GA):
                    nc.vector.scalar_tensor_tensor(
                        z[:, offA[ji]:offA[ji] + Tp[j]],
                        e_A[a][:, offA[ji]:offA[ji] + Tp[j]],
                        s_sb[:, a * BL + j:a * BL + j + 1],
                        qbcA[:, offA[ji]:offA[ji] + Tp[j]],
                        ALU.add, ALU.add)
                tv = loop_z.tile([128, LA], BF16, name="tvA", tag="tvA")
                nc.scalar.activation(tv[:], z[:], AF.Tanh,
                                     scale=wfb_colf[:, a:a + 1])
                for ji, j in enumerate(GA):
                    kw = dict(start=(a == 0), stop=(a == AC - 1),
                              skip_group_check=True)
                    if j:
                        kw["tile_position"] = (0, 32 * j)
                    for n0 in range(0, Tp[j], 512):
                        n1 = min(n0 + 512, Tp[j])
                        nc.tensor.matmul(Eps_A[32 * j:32 * j + 1, n0:n1],
                                         vT_col[:, a:a + 1],
                                         tv[:, offA[ji] + n0:offA[ji] + n1],
                                         **kw)
                # interleave: xemb prefetch + W_hh part of NEXT step's gates
                if a == 0:
                    gnext = gall_tiles[(par + 1) % 2]
                    xet = loop_sb.tile([128, GC * BL], BF16, name="xet",
                                       tag="xet")
                    nc.sync.dma_start(xet[:], xembT_d[:, bass.ds(t4 + BL, BL)]
                                      .rearrange("(c p) b -> p c b", p=128))
                    nc.tensor.matmul(gnext[:, 0:GC * BL], ident[:, :],
                                     xet[:, :], start=True, stop=False,
                                     skip_group_check=True)
                else:
                    gnext = gall_tiles[(par + 1) % 2]
                emit_ghh(gnext, a * (GC // AC), (a + 1) * (GC // AC))
                if a == 0:
                    # zoneout prefactors for the next step
                    nc.vector.tensor_scalar(c05[:], c_st[:], ZC, None, ALU.mult)
                    nc.vector.tensor_scalar(h05[:], h_bf[:], ZH, None, ALU.mult)

            # ---- group A tail (overlaps group B stretch) ----
            EnA = loop_se.tile([128, TpA], BF16, name="EnA", tag="EnA")
            nc.scalar.activation(EnA[:], Eps_A[:], AF.Exp)
            w1A = loop_se.tile([128, TpA], BF16, name="w1A", tag="w1A")
            seA = loop_se.tile([128, 1], F32, name="seA", tag="seA")
            nc.vector.scalar_tensor_tensor(w1A[0:96, :], EnA[0:96, :], 1.0,
                                           mask01[0:96, 0:TpA],
                                           ALU.mult, ALU.mult,
                                           accum_out=seA[0:96, :])
            rseA = loop_se.tile([128, 1], F32, name="rseA", tag="rseA")
            nc.vector.reciprocal(rseA[0:96, :], seA[0:96, :])
            nc.vector.tensor_scalar(w_att[0:96, 0:TpA], w1A[0:96, :],
                                    rseA[0:96, :], None, ALU.mult)
            nc.vector.scalar_tensor_tensor(EnA[0:96, :], w1A[0:96, :],
                                           rseA[0:96, :], finv[0:96, 0:TpA],
                                           ALU.mult, ALU.mult)
            nc.vector.tensor_tensor(accum_bf[0:96, 0:TpA], accum_bf[0:96, 0:TpA],
                                    EnA[0:96, :], ALU.add)
            for j in GA:
                nc.sync.dma_start(qd[0:1, j * 1024:(j + 1) * 1024],
                                  accum_bf[32 * j:32 * j + 1, :])
            for ji, j in enumerate(GA):
                nc.sync.dma_start(
                    qbcA[:, offA[ji]:offA[ji] + Tp[j]],
                    qd[0:1, j * 1024:j * 1024 + Tp[j]].partition_broadcast(128))

            # ---- group B stretch, interleaved with group A transposes and
            #      ctx matvecs ----
            for a in range(AC):
                zb = loop_z.tile([128, LB], BF16, name="zB", tag="zB")
                nc.vector.scalar_tensor_tensor(
                    zb[:, 0:LB], e_B[a][:, 0:LB],
                    s_sb[:, a * BL + 3:a * BL + 4],
                    qbcB[:, 0:LB], ALU.add, ALU.add)
                tvb = loop_z.tile([128, LB], BF16, name="tvB", tag="tvB")
                nc.scalar.activation(tvb[:], zb[:], AF.Tanh,
                                     scale=wfb_colf[:, a:a + 1])
                kw = dict(start=(a == 0), stop=(a == AC - 1),
                          skip_group_check=True, tile_position=(0, 96))
                for n0 in range(0, LB, 512):
                    n1 = min(n0 + 512, LB)
                    nc.tensor.matmul(Eps_B[96:97, n0:n1], vT_col[:, a:a + 1],
                                     tvb[:, n0:n1], **kw)
                # group A transpose + ctx matvec for t-chunk c = a
                c = a
                if c < TC[0]:
                    wtp = loop_ps2.tile([128, 96], BF16, name="wtp", tag="wtp")
                    nc.tensor.transpose(wtp[:, 0:96],
                                        w_att[0:96, c * 128:(c + 1) * 128],
                                        ident[0:96, 0:96])
                    nsl = sum(1 for j in GA if TC[j] > c)
                    nc.vector.tensor_copy(wts[:, c * BL:c * BL + nsl],
                                          wtp[:, 0:32 * nsl:32])
                    for j in GA:
                        if c >= TC[j]:
                            continue
                        for dk in range(DC):
                            nc.tensor.matmul(
                                sc_ps[:, COFF + dk * BL + j:COFF + dk * BL + j + 1],
                                enc_all[j][:, c * 512 + dk * 128:
                                           c * 512 + (dk + 1) * 128],
                                wts[:, c * BL + j:c * BL + j + 1],
                                start=False, stop=(c == TC[j] - 1),
                                skip_group_check=True)

            # ---- group B tail (serial) ----
            EnB = loop_se.tile([128, LB], BF16, name="EnB", tag="EnB")
            nc.scalar.activation(EnB[:], Eps_B[:], AF.Exp)
            w1B = loop_se.tile([128, LB], BF16, name="w1B", tag="w1B")
            seB = loop_se.tile([128, 1], F32, name="seB", tag="seB")
            nc.vector.scalar_tensor_tensor(w1B[96:128, :], EnB[96:128, :], 1.0,
                                           mask01[96:128, 0:LB],
                                           ALU.mult, ALU.mult,
                                           accum_out=seB[96:128, :])
            rseB = loop_se.tile([128, 1], F32, name="rseB", tag="rseB")
            nc.vector.reciprocal(rseB[96:128, :], seB[96:128, :])
            nc.vector.tensor_scalar(w_att[96:128, 0:LB], w1B[96:128, :],
                                    rseB[96:128, :], None, ALU.mult)
            nc.vector.scalar_tensor_tensor(EnB[96:128, :], w1B[96:128, :],
                                           rseB[96:128, :], finv[96:128, 0:LB],
                                           ALU.mult, ALU.mult)
            nc.vector.tensor_tensor(accum_bf[96:128, 0:LB],
                                    accum_bf[96:128, 0:LB],
                                    EnB[96:128, :], ALU.add)
            nc.sync.dma_start(qd[0:1, 3 * 1024:4 * 1024],
                              accum_bf[96:97, :])
            nc.sync.dma_start(
                qbcB[:, 0:LB],
                qd[0:1, 3 * 1024:3 * 1024 + LB].partition_broadcast(128))

            for c in range(TC[3]):
                wtp = loop_ps2.tile([128, 96], BF16, name="wtp", tag="wtp")
                nc.tensor.transpose(wtp[:, 0:32],
                                    w_att[96:128, c * 128:(c + 1) * 128],
                                    ident[96:128, 96:128],
                                    tile_position=(96, 0))
                nc.vector.tensor_copy(wts[:, c * BL + 3:c * BL + 4],
                                      wtp[:, 0:1])
                for dk in range(DC):
                    nc.tensor.matmul(
                        sc_ps[:, COFF + dk * BL + 3:COFF + dk * BL + 4],
                        enc_all[3][:, c * 512 + dk * 128:c * 512 + (dk + 1) * 128],
                        wts[:, c * BL + 3:c * BL + 4],
                        start=False, stop=(c == TC[3] - 1),
                        skip_group_check=True)

            nc.vector.tensor_copy(ctxT_sb[:], sc_ps[:, COFF:COFF + DC * BL])
            nc.vector.tensor_scalar(ctx_q[:], sc_ps[:, COFF:COFF + DC * BL], SX, None, ALU.mult)
            dst = cstk_d[:, bass.ds(t4, BL)].rearrange("(c p) b -> p c b", p=128)
            nc.sync.dma_start(dst, ctxT_sb[:])

        assert UNROLL % 2 == 0, "gall parity needs even UNROLL"
        with tc.For_i(0, _nsteps * BL, UNROLL * BL,
                      hint_engines=(ET.PE, ET.Activation, ET.DVE, ET.SP)) as t4:
            for s in range(UNROLL):
                step_body(t4 + s * BL, s)

        if dbg_d is not None:
            nc.sync.dma_start(dbg_d[:, 0:1024], w_att[:, :])
            nc.sync.dma_start(dbg_d[:, 1024:1024 + HC * BL], h_bf[:, :])
            nc.sync.dma_start(dbg_d[:, 1056:1056 + TCmax * BL], wts[:, :])
            pass

        # ============ READOUT ============
        inner.close()
        post_sb = ctx.enter_context(tc.tile_pool(name="post_sb", bufs=1))
        post_st = ctx.enter_context(tc.tile_pool(name="post_st", bufs=2))
        post_ps = ctx.enter_context(tc.tile_pool(name="post_ps", bufs=2,
                                                 space="PSUM"))
        trash_holder.append(post_ps.tile([128, 128], BF16, name="trash_post"))

        xro = []
        for k in range(HC):
            tl = post_sb.tile([128, NS], BF16, name=f"xh{k}", tag=f"xh{k}")
            nc.sync.dma_start(tl[:], hstk_d[k * 128:(k + 1) * 128, :])
            xro.append(tl)
        for k in range(EC):
            tl = post_sb.tile([128, NS], BF16, name=f"xe{k}", tag=f"xe{k}")
            nc.sync.dma_start(tl[:], embT_d[k * 128:(k + 1) * 128, :])
            xro.append(tl)
        for k in range(DC):
            tl = post_sb.tile([128, NS], BF16, name=f"xc{k}", tag=f"xc{k}")
            nc.sync.dma_start(tl[:], cstk_d[k * 128:(k + 1) * 128, :])
            xro.append(tl)
        W_roe = [post_sb.tile([128, RO // 2], BF16, name=f"wre{k}", tag=f"wre{k}")
                 for k in range(XROC)]
        W_roo = [post_sb.tile([128, RO // 2], BF16, name=f"wro{k}", tag=f"wro{k}")
                 for k in range(XROC)]
        for k in range(XROC):
            nc.sync.dma_start(W_roe[k][:], W_roT_e_d[k * 128:(k + 1) * 128, :])
            nc.sync.dma_start(W_roo[k][:], W_roT_o_d[k * 128:(k + 1) * 128, :])
        b_ro_e = post_sb.tile([128, ROC], F32)
        nc.sync.dma_start(b_ro_e[:], b_ro_e_d[:, :])
        b_ro_o = post_sb.tile([128, ROC], F32)
        nc.sync.dma_start(b_ro_o[:], b_ro_o_d[:, :])
        b_out_col = post_sb.tile([128, VC], F32)
        nc.sync.dma_start(b_out_col[:], b_out_d[:, :])
        pe_touch(xro[0][:, 0:128])
        pe_touch(W_roe[0][:, 0:128])
        pe_touch(W_roo[0][:, 0:128])

        maxo = []
        for oc in range(ROC):
            Re = post_ps.tile([128, NS], F32, name="Re", tag="Re")
            for k in range(XROC):
                nc.tensor.matmul(Re[:], W_roe[k][:, oc * 128:(oc + 1) * 128],
                                 xro[k][:], start=(k == 0), stop=(k == XROC - 1))
            t1 = post_st.tile([128, NS], F32, name="t1", tag="t1")
            nc.scalar.activation(t1[:], Re[:], AF.Identity,
                                 bias=b_ro_e[:, oc:oc + 1], scale=1.0)
            Ro = post_ps.tile([128, NS], F32, name="Ro", tag="Re")
            for k in range(XROC):
                nc.tensor.matmul(Ro[:], W_roo[k][:, oc * 128:(oc + 1) * 128],
                                 xro[k][:], start=(k == 0), stop=(k == XROC - 1))
            t2 = post_st.tile([128, NS], F32, name="t2", tag="t2")
            nc.scalar.activation(t2[:], Ro[:], AF.Identity,
                                 bias=b_ro_o[:, oc:oc + 1], scale=1.0)
            mo = post_sb.tile([128, NS], BF16, name=f"mo{oc}", tag=f"mo{oc}")
            nc.vector.tensor_tensor(mo[:], t1[:], t2[:], ALU.max)
            maxo.append(mo)

        wo_pool = ctx.enter_context(tc.tile_pool(name="wo", bufs=6))
        first = True
        for vc in range(VC):
            wo = [wo_pool.tile([128, 128], BF16, name=f"wo{vc}_{k}", tag=f"wok{k}")
                  for k in range(ROC)]
            for k in range(ROC):
                nc.sync.dma_start(wo[k][:],
                                  W_outT_d[k * 128:(k + 1) * 128,
                                           vc * 128:(vc + 1) * 128])
            if first:
                pe_touch(wo[0][:, 0:128])
                pe_touch(maxo[0][:, 0:128])
                first = False
            L = post_ps.tile([128, NS], F32, name="L", tag="L")
            for k in range(ROC):
                nc.tensor.matmul(L[:], wo[k][:], maxo[k][:],
                                 start=(k == 0), stop=(k == ROC - 1))
            lo = post_st.tile([128, NS], F32, name="lo", tag="lo")
            nc.scalar.activation(lo[:], L[:], AF.Identity,
                                 bias=b_out_col[:, vc:vc + 1], scale=1.0)
            nc.sync.dma_start(out_d[vc * 128:(vc + 1) * 128, :], lo[:])

    return nc


def check_waits(nc, cap_note=""):
    bad = []
    for fn in nc.m.functions:
        for bb in fn.blocks:
            for inst in bb.instructions:
                c = inst.concise()
                nw = c.count("wait:")
                eng = c.split()[0] if c.split() else "?"
                if nw >= 2 and eng in ("PE", "ACT", "DVE", "PL"):
                    bad.append((nw, c[:180]))
    for nw, c in bad:
        print("WAITS", nw, c)
    return bad


def _prep_core(inputs, order, Tp, core):
    enc = np.asarray(inputs["encoder_outputs"], np.float32)
    labels = np.asarray(inputs["labels"])
    lens = np.asarray(inputs["enc_seq_len"], np.int64)
    embed = np.asarray(inputs["embed"], np.float32)

    bidx = [int(order[j * NCORE + core]) for j in range(BL)]
    m = {}
    for j in range(BL):
        b = bidx[j]
        ep = np.zeros((Tp[j], D), np.float32)
        ep[:T] = enc[b, :Tp[j] if Tp[j] <= T else T]
        m[f"enc_td{j}"] = _bf(ep)
        m[f"encT{j}"] = _bf(ep.T)
    emb = np.zeros((BL, S, E), np.float32)
    for j in range(BL):
        b = bidx[j]
        emb[j, 1:] = embed[labels[b, :S - 1].astype(np.int64)]
    embT = emb.transpose(2, 1, 0).reshape(E, NS)
    m["embT"] = _bf(embT)
    mask01 = np.zeros((BL, 1024), np.float32)
    for j in range(BL):
        mask01[j, :int(lens[bidx[j]])] = 1.0
    m["mask01"] = _bf(mask01)
    return m, bidx


def kernel(**inputs):
    lens = np.asarray(inputs["enc_seq_len"], np.int64)
    order = np.argsort(-lens, kind="stable")
    Tp = []
    for j in range(BL):
        mx = max(int(lens[order[j * NCORE + i]]) for i in range(NCORE))
        Tp.append(min(1024, ((mx + 127) // 128) * 128))

    perm = _gate_perm()
    W_ih = np.asarray(inputs["W_ih"], np.float32)[perm]
    W_hh = np.asarray(inputs["W_hh"], np.float32)[perm]
    b_sum = (np.asarray(inputs["b_ih"], np.float32)
             + np.asarray(inputs["b_hh"], np.float32))[perm]
    wfb = np.asarray(inputs["W_fb"], np.float32)[:, 0]
    wfb_safe = np.where(wfb >= 0, np.maximum(wfb, 1e-3),
                        np.minimum(wfb, -1e-3))
    shared = {
        "W_combT": _w8(np.concatenate([W_ih[:, E:].T, W_hh.T], 0)),
        "W_ih_embT": _bf(W_ih[:, :E].T),
        "W_encT": _bf(np.asarray(inputs["W_enc"], np.float32).T
                      / wfb_safe[None, :]),
        "W_sT": _bf(np.asarray(inputs["W_s"], np.float32).T
                    / wfb_safe[None, :]),
        "wfert_col": _bf(np.asarray(inputs["W_fert"],
                                    np.float32).reshape(DC, 128).T),
        "vT_col": _bf(np.asarray(inputs["v_att"], np.float32).reshape(AC, 128).T),
        "wfb_colf": np.ascontiguousarray(
            wfb_safe.reshape(AC, 128).T.astype(np.float32)),
        "b_enc_col": np.ascontiguousarray(
            (np.asarray(inputs["b_enc"], np.float32) / wfb_safe)
            .reshape(AC, 128).T),
        "b_comb": np.ascontiguousarray(
            (b_sum * SG).reshape(GC, 128).T),
        "W_roT_e": _bf(np.asarray(inputs["W_ro"], np.float32)[0::2].T),
        "W_roT_o": _bf(np.asarray(inputs["W_ro"], np.float32)[1::2].T),
        "b_ro_e": np.ascontiguousarray(
            np.asarray(inputs["b_ro"], np.float32)[0::2].reshape(ROC, 128).T),
        "b_ro_o": np.ascontiguousarray(
            np.asarray(inputs["b_ro"], np.float32)[1::2].reshape(ROC, 128).T),
        "W_outT": _bf(np.asarray(inputs["W_out"], np.float32).T),
        "b_out_col": np.ascontiguousarray(
            np.asarray(inputs["b_out"], np.float32).reshape(VC, 128).T),
    }

    in_maps = []
    bidx_all = []
    for c in range(NCORE):
        m, bidx = _prep_core(inputs, order, Tp, c)
        m.update(shared)
        in_maps.append(m)
        bidx_all.append(bidx)

    nc = build_nc(Tp)
    nc.finalize()
    from concourse.bass_utils import run_bass_kernel_spmd
    trace = bool(os.environ.get("BASS_KERNEL_TRACE"))
    res = run_bass_kernel_spmd(nc, in_maps, core_ids=list(range(NCORE)),
                               trace=trace)
    global LAST_EXEC_NS, LAST_OUTS, LAST_META
    LAST_EXEC_NS = res.exec_time_ns
    outs = res.results
    LAST_OUTS = outs
    LAST_META = (order, Tp, bidx_all)

    logits = np.zeros((B, S, V), np.float32)
    for c in range(NCORE):
        o = outs[c]["out"].reshape(V, S, BL)
        for j in range(BL):
            logits[bidx_all[c][j]] = o[:, :, j].T
    return logits


if __name__ == "__main__":
    nc = build_nc([1024, 896, 768, 640])
    bad = check_waits(nc)
    print(f"{len(bad)} instructions with >=2 waits")


# revision 36
# speedup vs baseline: 1.2913x; 1.0422x over previous
"""Attention-LSTM decoder (B=32, T=1000, S=100, D=512, A=1024, H=1024,
E=640, V=10240, P=1024) on 8 trn2 NeuronCores.

Sharding: data-parallel over batch, 4 batches per core (one per "slot").
Batches are sorted by enc_seq_len; slot j holds ranks [j*8:(j+1)*8] so the
padded time extent Tp[j] (multiple of 128) is shared by all 8 cores and the
SPMD graph is identical across cores.

v3 design (vs. v2 baseline):
  - attention slots split into group A = slots {0,1,2} and group B = {3}.
    Group A's softmax / transpose / ctx-matvec tail executes underneath
    group B's tanh stretch; only B's short tail is serial.
  - tanh merged per (group, a-chunk): s_t/wfb is folded into the z-add via
    scalar_tensor_tensor with the per-partition scalar read directly from
    the s PSUM tile (host pre-divides W_s rows by wfb), so one ACT
    instruction covers all slots of a group.
  - softmax: exp -> one STT that applies the {0,1} mask AND emits the row
    sums via accum_out (no tensor_reduce); w*finv fused the same way.
  - gates accumulate in a single PSUM tile: W_hh part prefetched during the
    previous stretch, ctx part + xemb (via identity matmul) appended, and
    the activations read PSUM directly.
  - LSTM pointwise lowered to 5 STT + 1 TT + 1 TS using 2*sigmoid(x) =
    tanh(x/2) + 1; zoneout blends use pre-scaled c05/h05 computed during
    the previous stretch.
"""
import sys

sys.path.insert(0, "/opt/trn_rl_repo")

import os
import numpy as np
import ml_dtypes
from contextlib import ExitStack

import concourse.bass as bass
import concourse.tile as tile
import concourse.mybir as mybir
from concourse import bacc
from concourse.masks import make_identity

DT = mybir.dt
F32 = DT.float32
BF16 = DT.bfloat16
FP8 = DT.float8e4
AF = mybir.ActivationFunctionType
ALU = mybir.AluOpType
ET = mybir.EngineType

B, T, S = 32, 1000, 100
D, A, H, E, V, RO = 512, 1024, 1024, 640, 10240, 1024
ZH, ZC = 0.05, 0.15
NCORE = 8
BL = B // NCORE          # 4 batches (slots) per core
NS = S * BL              # 400 step-batch columns
GC = 4 * H // 128        # 32 gate chunks
HC = H // 128            # 8
AC = A // 128            # 8
DC = D // 128            # 4
EC = E // 128            # 5
ROC = RO // 2 // 128     # 4 chunks per maxout half
VC = V // 128            # 80 vocab chunks
XROC = (H + E + D) // 128  # 17 readout K-chunks

USE_FP8 = os.environ.get("KBFP8", "1") != "0"
SW = 64.0 if USE_FP8 else 1.0    # weight scale
SX = 16.0 if USE_FP8 else 1.0    # moving (h/ctx) scale
SG = SW * SX                      # psum scale for gates
WDT = FP8 if USE_FP8 else BF16

GA = (0, 1, 2)  # group A slots
GB = (3,)       # group B slots

bf16 = ml_dtypes.bfloat16
f8 = ml_dtypes.float8_e4m3
LAST_EXEC_NS = None
LAST_OUTS = None
LAST_META = None


def _bf(a):
    return np.ascontiguousarray(np.asarray(a, dtype=np.float32)).astype(bf16)


def _w8(a):
    a = np.asarray(a, dtype=np.float32) * SW
    return np.ascontiguousarray(a).astype(f8 if USE_FP8 else bf16)


# gate-permutation: reference gate order is [i|f|g|o]; we reorder rows to
# [i|f|o|g] so the three sigmoids are contiguous.
def _gate_perm():
    idx = np.arange(4 * H)
    return np.concatenate([idx[0:2 * H], idx[3 * H:4 * H], idx[2 * H:3 * H]])


def build_nc(Tp, debug=False):
    TC = [t // 128 for t in Tp]
    TCmax = max(TC)
    offA = [0, Tp[0], Tp[0] + Tp[1]]       # segment offsets in group-A tiles
    LA = Tp[0] + Tp[1] + Tp[2]
    LB = Tp[3]
    TpA = Tp[0]                            # group-A col extent (max of group)
    nc = bacc.Bacc("TRN2", target_bir_lowering=False)

    def param(name, shape, dt=BF16):
        return nc.declare_dram_parameter(name, list(shape), dt, isOutput=False)

    enc_td = [param(f"enc_td{j}", [Tp[j], D]) for j in range(BL)]
    encT = [param(f"encT{j}", [D, Tp[j]]) for j in range(BL)]
    embT_d = param("embT", [E, NS])
    W_combT_d = param("W_combT", [D + H, 4 * H], WDT)
    W_ih_embT_d = param("W_ih_embT", [E, 4 * H])
    W_encT_d = param("W_encT", [D, A])
    W_sT_d = param("W_sT", [H, A])                 # bf16 W_s.T
    wfert_col_d = param("wfert_col", [128, DC])
    vT_col_d = param("vT_col", [128, AC])
    wfb_colf_d = param("wfb_colf", [128, AC], F32)
    b_enc_col_d = param("b_enc_col", [128, AC], F32)
    b_comb_d = param("b_comb", [128, GC], F32)     # pre-scaled by SG on host
    mask01_d = param("mask01", [BL, 1024])         # {0,1} rows
    W_roT_e_d = param("W_roT_e", [H + E + D, RO // 2])
    W_roT_o_d = param("W_roT_o", [H + E + D, RO // 2])
    b_ro_e_d = param("b_ro_e", [128, ROC], F32)
    b_ro_o_d = param("b_ro_o", [128, ROC], F32)
    W_outT_d = param("W_outT", [RO // 2, V])
    b_out_d = param("b_out_col", [128, VC], F32)
    out_d = nc.declare_dram_parameter("out", [V, NS], F32, isOutput=True)

    qd = nc.dram_tensor("qd", [1, BL * 1024], BF16)
    dbg_d = nc.declare_dram_parameter("dbgt", [128, 1088], BF16,
                                      isOutput=True) \
        if os.environ.get("KBDBG") else None
    dbg2_d = nc.declare_dram_parameter("dbgt2", [128, 576], F32,
                                       isOutput=True) \
        if os.environ.get("KBDBG") else None
    hstk_d = nc.dram_tensor("hstk", [H, NS], BF16)
    cstk_d = nc.dram_tensor("cstk", [D, NS], BF16)
    xembT_d = nc.dram_tensor("xembT", [4 * H, NS + BL], BF16)

    with ExitStack() as ctx:
        tc = ctx.enter_context(tile.TileContext(nc))

        # ---------------- persistent pools ----------------
        persist = ctx.enter_context(tc.tile_pool(name="persist", bufs=1))
        ident = persist.tile([128, 128], BF16)
        make_identity(nc, ident[:])
        vT_col = persist.tile([128, AC], BF16)
        nc.sync.dma_start(vT_col[:], vT_col_d[:, :])
        wfb_colf = persist.tile([128, AC], F32)
        nc.sync.dma_start(wfb_colf[:], wfb_colf_d[:, :])
        wfert_col = persist.tile([128, DC], BF16)
        nc.sync.dma_start(wfert_col[:], wfert_col_d[:, :])
        b_enc_col = persist.tile([128, AC], F32)
        nc.sync.dma_start(b_enc_col[:], b_enc_col_d[:, :])
        b_comb = persist.tile([128, GC], F32)
        nc.sync.dma_start(b_comb[:], b_comb_d[:, :])
        mask01 = persist.tile([128, 1024], BF16)
        nc.vector.memset(mask01[:], 0.0)
        # col 0 = 1 on every row so dead-row softmax sums stay finite
        nc.vector.memset(mask01[:, 0:1], 1.0)
        for j in range(BL):
            nc.sync.dma_start(mask01[32 * j:32 * j + 1, :], mask01_d[j:j + 1, :])

        h_bf = persist.tile([128, HC * BL], BF16)
        h_q = persist.tile([128, HC * BL], WDT)
        c_st = persist.tile([128, HC * BL], F32)
        c05 = persist.tile([128, HC * BL], F32)
        h05 = persist.tile([128, HC * BL], F32)
        ctxT_sb = persist.tile([128, DC * BL], BF16)
        ctx_q = persist.tile([128, DC * BL], WDT)
        accum_bf = persist.tile([128, 1024], BF16)  # rows {0,32,64,96}
        w_att = persist.tile([128, 1024], BF16)
        finv = persist.tile([128, 1024], BF16)     # rows {0,32,64,96}, x0.5
        wts = persist.tile([128, TCmax * BL], BF16)
        s_sb = persist.tile([128, AC * BL], F32)
        zrow = persist.tile([1, 128], BF16)
        nc.vector.memset(zrow[:], 0.0)
        zpad = persist.tile([128, GC * BL], BF16)
        nc.vector.memset(zpad[:], 0.0)
        nc.sync.dma_start(
            xembT_d[:, NS:NS + BL].rearrange("(c p) b -> p c b", p=128),
            zpad[:])
        for t_ in (h_bf, h_q, c_st, c05, h05, ctxT_sb, ctx_q, accum_bf,
                   w_att, finv, wts):
            nc.vector.memset(t_[:], 0.0)

        inner = ctx.enter_context(ExitStack())
        e_pool = inner.enter_context(tc.tile_pool(name="e", bufs=1))
        e_A = [e_pool.tile([128, LA], BF16, name=f"eA{a}", tag=f"eA{a}")
               for a in range(AC)]
        e_B = [e_pool.tile([128, LB], BF16, name=f"eB{a}", tag=f"eB{a}")
               for a in range(AC)]

        trash_holder = []

        def pe_touch(ap):
            # phase-scoped trash tile (pre/post only; fp8 touches are no-ops)
            if ap.dtype not in (BF16,) or not trash_holder:
                return
            trash_ps = trash_holder[0]
            p = ap.shape[0]
            nc.tensor.transpose(trash_ps[0:min(ap.shape[1], 128), 0:p],
                                ap[:, 0:min(ap.shape[1], 128)], ident[0:p, 0:p])

        # ============ PRECOMPUTE PHASE ============
        with ExitStack() as pre:
            pre_sb = pre.enter_context(tc.tile_pool(name="pre_sb", bufs=1))
            pre_st = pre.enter_context(tc.tile_pool(name="pre_st", bufs=2))
            pre_ps = pre.enter_context(tc.tile_pool(name="pre_ps", bufs=1,
                                                    space="PSUM"))
            trash_holder.append(pre_ps.tile([128, 128], BF16, name="trash_pre"))

            W_encT = [pre_sb.tile([128, A], BF16, name=f"wenc{k}", tag=f"we{k}")
                      for k in range(DC)]
            for k in range(DC):
                nc.sync.dma_start(W_encT[k][:], W_encT_d[k * 128:(k + 1) * 128, :])
            pe_touch(W_encT[0][:, 0:128])

            for j in range(BL):
                ercs = [pre_st.tile([128, Tp[j]], BF16, name=f"erc{j}{k}",
                                    tag=f"erc{k}") for k in range(DC)]
                for k in range(DC):
                    nc.sync.dma_start(ercs[k][:], encT[j][k * 128:(k + 1) * 128, :])
                    pe_touch(ercs[k][:, 0:128])
                for a in range(AC):
                    pe2 = pre_ps.tile([128, 1024], F32, name="pe_e2", tag="pe_e2")
                    for k in range(DC):
                        for n0 in range(0, Tp[j], 512):
                            n1 = min(n0 + 512, Tp[j])
                            nc.tensor.matmul(pe2[:, n0:n1],
                                             W_encT[k][:, a * 128:(a + 1) * 128],
                                             ercs[k][:, n0:n1],
                                             start=(k == 0), stop=(k == DC - 1))
                    if j in GA:
                        dst = e_A[a][:, offA[j]:offA[j] + Tp[j]]
                    else:
                        dst = e_B[a][:, 0:Tp[j]]
                    nc.scalar.activation(dst, pe2[:, 0:Tp[j]],
                                         AF.Identity,
                                         bias=b_enc_col[:, a:a + 1], scale=1.0)
                pf = pre_ps.tile([1, 1024], F32, name="pf", tag="pf")
                for k in range(DC):
                    for n0 in range(0, Tp[j], 512):
                        n1 = min(n0 + 512, Tp[j])
                        nc.tensor.matmul(pf[0:1, n0:n1], wfert_col[:, k:k + 1],
                                         ercs[k][:, n0:n1],
                                         start=(k == 0), stop=(k == DC - 1))
                # finv = 0.5*sigmoid(x) = 0.25*tanh(0.5x) + 0.25  (no table sw)
                fstage = pre_st.tile([1, 1024], F32, name="fstage", tag="fstage")
                nc.scalar.activation(fstage[0:1, 0:Tp[j]], pf[0:1, 0:Tp[j]],
                                     AF.Tanh, scale=0.5)
                fst2 = pre_st.tile([1, 1024], BF16, name="fst2", tag="fst2")
                nc.vector.tensor_scalar(fst2[0:1, 0:Tp[j]], fstage[0:1, 0:Tp[j]],
                                        0.25, 0.25, ALU.mult, ALU.add)
                nc.sync.dma_start(finv[32 * j:32 * j + 1, 0:Tp[j]],
                                  fst2[0:1, 0:Tp[j]])

            embT_sb = [pre_sb.tile([128, NS], BF16, name=f"embs{k}", tag=f"em{k}")
                       for k in range(EC)]
            for k in range(EC):
                nc.sync.dma_start(embT_sb[k][:], embT_d[k * 128:(k + 1) * 128, :])
            W_ie = [pre_sb.tile([128, 4 * H], BF16, name=f"wie{k}", tag=f"wi{k}")
                    for k in range(EC)]
            for k in range(EC):
                nc.sync.dma_start(W_ie[k][:], W_ih_embT_d[k * 128:(k + 1) * 128, :])
            pe_touch(W_ie[0][:, 0:128])
            pe_touch(embT_sb[0][:, 0:128])
            for g in range(GC):
                px = pre_ps.tile([128, NS], F32, name="px", tag="pe_e2")
                for k in range(EC):
                    nc.tensor.matmul(px[:], W_ie[k][:, g * 128:(g + 1) * 128],
                                     embT_sb[k][:], start=(k == 0),
                                     stop=(k == EC - 1))
                # xemb scaled by SG, bias pre-scaled on host
                stg = pre_st.tile([128, NS], BF16, name="xstg", tag="xstg")
                nc.scalar.activation(stg[:], px[:], AF.Identity,
                                     bias=b_comb[:, g:g + 1], scale=SG)
                nc.sync.dma_start(xembT_d[g * 128:(g + 1) * 128, 0:NS], stg[:])

        trash_holder.clear()

        # ============ WEIGHTS (loop phase) ============
        ppsum = inner.enter_context(tc.tile_pool(name="ppsum", bufs=1,
                                                 space="PSUM"))
        Eps_A = ppsum.tile([128, TpA], F32)
        Eps_B = ppsum.tile([128, LB], F32)
        gall_tiles = [ppsum.tile([128, GC * BL], F32, name=f"gall{i}",
                                 tag=f"gall{i}") for i in range(2)]
        w_pool = inner.enter_context(tc.tile_pool(name="w", bufs=1))
        W_comb = [w_pool.tile([128, 4 * H], WDT, name=f"wc{k}", tag=f"wc{k}")
                  for k in range(DC + HC)]
        for k in range(DC + HC):
            nc.sync.dma_start(W_comb[k][:], W_combT_d[k * 128:(k + 1) * 128, :])
        W_sT = [w_pool.tile([128, A], BF16, name=f"ws{k}", tag=f"ws{k}")
                for k in range(HC)]
        for k in range(HC):
            nc.sync.dma_start(W_sT[k][:], W_sT_d[k * 128:(k + 1) * 128, :])

        qbcA = None  # set below (persistent, loop-carried)
        loop_sb = inner.enter_context(tc.tile_pool(name="lsb", bufs=2))
        loop_z = inner.enter_context(tc.tile_pool(name="lz", bufs=2))
        loop_q = inner.enter_context(tc.tile_pool(name="lq", bufs=2))
        loop_se = inner.enter_context(tc.tile_pool(name="lse", bufs=1))
        enc_pool = inner.enter_context(tc.tile_pool(name="encp", bufs=1))
        loop_ps2 = inner.enter_context(tc.tile_pool(name="lps2", bufs=1,
                                                    space="PSUM"))

        # enc resident in SBUF for the whole loop (identical every step)
        enc_all = []
        for j in range(BL):
            ea = enc_pool.tile([128, TC[j] * 512], BF16, name=f"ea{j}",
                               tag=f"ea{j}")
            nc.sync.dma_start(
                ea[:], enc_td[j][:, :].rearrange("(c p) d -> p c d", p=128))
            enc_all.append(ea)

        nc.vector.memset(Eps_A[:], 0.0)
        nc.vector.memset(Eps_B[:], 0.0)

        def emit_ghh(gall, g0, g1):
            # the gall tile is opened by a full-tile xet identity-matmul
            # (start=True); everything after accumulates onto written bytes.
            for g in range(g0, g1):
                for k in range(HC):
                    nc.tensor.matmul(gall[:, g * BL:(g + 1) * BL],
                                     W_comb[DC + k][:, g * 128:(g + 1) * 128],
                                     h_q[:, k * BL:(k + 1) * BL],
                                     start=False, stop=False,
                                     skip_group_check=True)

        # prologue: xemb(0) opens the psum tile, then W_hh part with h=0
        gall_cur = gall_tiles[0]
        xet0 = loop_sb.tile([128, GC * BL], BF16, name="xet", tag="xet")
        nc.sync.dma_start(xet0[:], xembT_d[:, 0:BL].rearrange(
            "(c p) b -> p c b", p=128))
        nc.tensor.matmul(gall_cur[:, 0:GC * BL], ident[:, :], xet0[:, :],
                         start=True, stop=False, skip_group_check=True)
        emit_ghh(gall_cur, 0, GC)
        # initial accum (=0) -> qd -> qbc (persistent loop-carried tiles)
        for j in range(BL):
            nc.sync.dma_start(qd[0:1, j * 1024:(j + 1) * 1024],
                              accum_bf[32 * j:32 * j + 1, :])
        qbcA = loop_q.tile([128, LA], BF16, name="qbcA", tag="qbcA")
        for ji, j in enumerate(GA):
            nc.sync.dma_start(
                qbcA[:, offA[ji]:offA[ji] + Tp[j]],
                qd[0:1, j * 1024:j * 1024 + Tp[j]].partition_broadcast(128))
        qbcB = loop_q.tile([128, LB], BF16, name="qbcB", tag="qbcB")
        nc.sync.dma_start(
            qbcB[:, 0:LB],
            qd[0:1, 3 * 1024:3 * 1024 + LB].partition_broadcast(128))

        # ============ STEP LOOP ============
        _nsteps = int(os.environ.get("KBSTEPS", S))
        UNROLL = int(os.environ.get("KBUNROLL", 4))

        last_sc = [None]
        scount = [0]

        def step_body(t4, par):
            gall = gall_tiles[par % 2]

            # ---- gates: ctx part (xemb + W_hh already accumulated) ----
            for g in range(GC):
                for k in range(DC):
                    nc.tensor.matmul(gall[:, g * BL:(g + 1) * BL],
                                     W_comb[k][:, g * 128:(g + 1) * 128],
                                     ctx_q[:, k * BL:(k + 1) * BL],
                                     start=False, stop=(g == GC - 1 and
                                                        k == DC - 1),
                                     skip_group_check=True)

            if dbg2_d is not None:
                sidx = scount[0]
                scount[0] += 1
                dtile = loop_sb.tile([128, GC * BL], F32, name="dtile",
                                     tag="dtile")
                nc.vector.tensor_copy(dtile[:], gall[:, :])
                nc.sync.dma_start(
                    dbg2_d[:, 64 + (sidx % 4) * 128:64 + (sidx % 4) * 128 + 128],
                    dtile[:])

            # ---- LSTM pointwise (fused) ----
            HB = HC * BL
            t_ifo = loop_sb.tile([128, 3 * HB], F32, name="t_ifo", tag="ti")
            nc.scalar.activation(t_ifo[:], gall[:, 0:3 * HB], AF.Tanh,
                                 scale=0.5 / SG)
            t_g = loop_sb.tile([128, HB], F32, name="t_g", tag="tg")
            nc.scalar.activation(t_g[:], gall[:, 3 * HB:], AF.Tanh,
                                 scale=1.0 / SG)
            av = loop_sb.tile([128, HB], F32, name="av", tag="av")
            nc.vector.scalar_tensor_tensor(av[:], t_ifo[:, HB:2 * HB], 1.0,
                                           c_st[:], ALU.add, ALU.mult)
            bv = loop_sb.tile([128, HB], F32, name="bv", tag="bv")
            nc.vector.scalar_tensor_tensor(bv[:], t_ifo[:, 0:HB], 1.0,
                                           t_g[:], ALU.add, ALU.mult)
            cn2 = loop_sb.tile([128, HB], F32, name="cn2", tag="cn2")
            nc.vector.tensor_tensor(cn2[:], av[:], bv[:], ALU.add)
            nc.vector.scalar_tensor_tensor(c_st[:], cn2[:], 0.5 * (1.0 - ZC),
                                           c05[:], ALU.mult, ALU.add)
            tcn = loop_sb.tile([128, HB], F32, name="tcn", tag="tcn")
            nc.scalar.activation(tcn[:], cn2[:], AF.Tanh, scale=0.5)
            hn2 = loop_sb.tile([128, HB], F32, name="hn2", tag="hn2")
            nc.vector.scalar_tensor_tensor(hn2[:], t_ifo[:, 2 * HB:3 * HB], 1.0,
                                           tcn[:], ALU.add, ALU.mult)
            nc.vector.scalar_tensor_tensor(h_bf[:], hn2[:], 0.5 * (1.0 - ZH),
                                           h05[:], ALU.mult, ALU.add)
            nc.vector.tensor_scalar(h_q[:], h_bf[:], SX, None, ALU.mult)
            dst = hstk_d[:, bass.ds(t4, BL)].rearrange("(c p) b -> p c b", p=128)
            nc.sync.dma_start(dst, h_bf[:])

            # ---- s' = h @ (W_s/wfb).T  (a-major so z can start early) ----
            # sc_ps holds s' in cols [0:32] and ctx in cols [32:48] (one
            # PSUM bank for both).
            COFF = AC * BL
            sc_ps = loop_ps2.tile([128, AC * BL + DC * BL], F32, name="sc_ps",
                                  tag="sc_ps")
            last_sc[0] = sc_ps
            nc.tensor.matmul(sc_ps[:, 0:COFF + DC * BL], zrow[0:1, 0:128],
                             zrow[0:1, 0:COFF + DC * BL],
                             start=True, stop=False, skip_group_check=True)
            for a in range(AC):
                for k in range(HC):
                    nc.tensor.matmul(sc_ps[:, a * BL:(a + 1) * BL],
                                     W_sT[k][:, a * 128:(a + 1) * 128],
                                     h_bf[:, k * BL:(k + 1) * BL],
                                     start=False,
                                     stop=(a == AC - 1 and k == HC - 1),
                                     skip_group_check=True)
                nc.vector.tensor_copy(s_sb[:, a * BL:(a + 1) * BL],
                                      sc_ps[:, a * BL:(a + 1) * BL])

            # ---- group A stretch ----
            for a in range(AC):
                z = loop_z.tile([128, LA], BF16, name="zA", tag="zA")
                for ji, j in enumerate(GA):
                    nc.vector.tensor_tensor(
                        z[:, offA[ji]:offA[ji] + Tp[j]],
                        e_A[a][:, offA[ji]:offA[ji] + Tp[j]],
                        qbcA[:, offA[ji]:offA[ji] + Tp[j]], ALU.add)
                tv = loop_z.tile([128, LA], BF16, name="tvA", tag="tvA")
                for ji, j in enumerate(GA):
                    nc.scalar.activation(tv[:, offA[ji]:offA[ji] + Tp[j]],
                                         z[:, offA[ji]:offA[ji] + Tp[j]],
                                         AF.Tanh,
                                         bias=s_sb[:, a * BL + j:a * BL + j + 1],
                                         scale=wfb_colf[:, a:a + 1])
                for ji, j in enumerate(GA):
                    kw = dict(start=(a == 0), stop=(a == AC - 1),
                              skip_group_check=True)
                    if j:
                        kw["tile_position"] = (0, 32 * j)
                    for n0 in range(0, Tp[j], 512):
                        n1 = min(n0 + 512, Tp[j])
                        nc.tensor.matmul(Eps_A[32 * j:32 * j + 1, n0:n1],
                                         vT_col[:, a:a + 1],
                                         tv[:, offA[ji] + n0:offA[ji] + n1],
                                         **kw)
                # interleave: xemb prefetch + W_hh part of NEXT step's gates
                if a == 0:
                    gnext = gall_tiles[(par + 1) % 2]
                    xet = loop_sb.tile([128, GC * BL], BF16, name="xet",
                                       tag="xet")
                    nc.sync.dma_start(xet[:], xembT_d[:, bass.ds(t4 + BL, BL)]
                                      .rearrange("(c p) b -> p c b", p=128))
                    nc.tensor.matmul(gnext[:, 0:GC * BL], ident[:, :],
                                     xet[:, :], start=True, stop=False,
                                     skip_group_check=True)
                else:
                    gnext = gall_tiles[(par + 1) % 2]
                emit_ghh(gnext, a * (GC // AC), (a + 1) * (GC // AC))
                if a == 0:
                    # zoneout prefactors for the next step
                    nc.vector.tensor_scalar(c05[:], c_st[:], ZC, None, ALU.mult)
                    nc.vector.tensor_scalar(h05[:], h_bf[:], ZH, None, ALU.mult)

            # ---- group A tail (overlaps group B stretch) ----
            EnA = loop_se.tile([128, TpA], BF16, name="EnA", tag="EnA")
            nc.scalar.activation(EnA[:], Eps_A[:], AF.Exp)
            w1A = loop_se.tile([128, TpA], BF16, name="w1A", tag="w1A")
            seA = loop_se.tile([128, 1], F32, name="seA", tag="seA")
            nc.vector.scalar_tensor_tensor(w1A[0:96, :], EnA[0:96, :], 1.0,
                                           mask01[0:96, 0:TpA],
                                           ALU.mult, ALU.mult,
                                           accum_out=seA[0:96, :])
            rseA = loop_se.tile([128, 1], F32, name="rseA", tag="rseA")
            nc.vector.reciprocal(rseA[0:96, :], seA[0:96, :])
            nc.vector.tensor_scalar(w_att[0:96, 0:TpA], w1A[0:96, :],
                                    rseA[0:96, :], None, ALU.mult)
            nc.vector.scalar_tensor_tensor(EnA[0:96, :], w1A[0:96, :],
                                           rseA[0:96, :], finv[0:96, 0:TpA],
                                           ALU.mult, ALU.mult)
            nc.vector.tensor_tensor(accum_bf[0:96, 0:TpA], accum_bf[0:96, 0:TpA],
                                    EnA[0:96, :], ALU.add)
            for j in GA:
                nc.sync.dma_start(qd[0:1, j * 1024:(j + 1) * 1024],
                                  accum_bf[32 * j:32 * j + 1, :])
            for ji, j in enumerate(GA):
                nc.sync.dma_start(
                    qbcA[:, offA[ji]:offA[ji] + Tp[j]],
                    qd[0:1, j * 1024:j * 1024 + Tp[j]].partition_broadcast(128))

            # ---- group B stretch, interleaved with group A transposes and
            #      ctx matvecs ----
            for a in range(AC):
                zb = loop_z.tile([128, LB], BF16, name="zB", tag="zB")
                nc.vector.tensor_tensor(zb[:, 0:LB], e_B[a][:, 0:LB],
                                        qbcB[:, 0:LB], ALU.add)
                tvb = loop_z.tile([128, LB], BF16, name="tvB", tag="tvB")
                nc.scalar.activation(tvb[:], zb[:], AF.Tanh,
                                     bias=s_sb[:, a * BL + 3:a * BL + 4],
                                     scale=wfb_colf[:, a:a + 1])
                kw = dict(start=(a == 0), stop=(a == AC - 1),
                          skip_group_check=True, tile_position=(0, 96))
                for n0 in range(0, LB, 512):
                    n1 = min(n0 + 512, LB)
                    nc.tensor.matmul(Eps_B[96:97, n0:n1], vT_col[:, a:a + 1],
                                     tvb[:, n0:n1], **kw)
                # group A transpose + ctx matvec for t-chunk c = a
                c = a
                if c < TC[0]:
                    wtp = loop_ps2.tile([128, 96], BF16, name="wtp", tag="wtp")
                    nc.tensor.transpose(wtp[:, 0:96],
                                        w_att[0:96, c * 128:(c + 1) * 128],
                                        ident[0:96, 0:96])
                    nsl = sum(1 for j in GA if TC[j] > c)
                    nc.vector.tensor_copy(wts[:, c * BL:c * BL + nsl],
                                          wtp[:, 0:32 * nsl:32])
                    for j in GA:
                        if c >= TC[j]:
                            continue
                        for dk in range(DC):
                            nc.tensor.matmul(
                                sc_ps[:, COFF + dk * BL + j:COFF + dk * BL + j + 1],
                                enc_all[j][:, c * 512 + dk * 128:
                                           c * 512 + (dk + 1) * 128],
                                wts[:, c * BL + j:c * BL + j + 1],
                                start=False, stop=(c == TC[j] - 1),
                                skip_group_check=True)

            # ---- group B tail (serial) ----
            EnB = loop_se.tile([128, LB], BF16, name="EnB", tag="EnB")
            nc.scalar.activation(EnB[:], Eps_B[:], AF.Exp)
            w1B = loop_se.tile([128, LB], BF16, name="w1B", tag="w1B")
            seB = loop_se.tile([128, 1], F32, name="seB", tag="seB")
            nc.vector.scalar_tensor_tensor(w1B[96:128, :], EnB[96:128, :], 1.0,
                                           mask01[96:128, 0:LB],
                                           ALU.mult, ALU.mult,
                                           accum_out=seB[96:128, :])
            rseB = loop_se.tile([128, 1], F32, name="rseB", tag="rseB")
            nc.vector.reciprocal(rseB[96:128, :], seB[96:128, :])
            nc.vector.tensor_scalar(w_att[96:128, 0:LB], w1B[96:128, :],
                                    rseB[96:128, :], None, ALU.mult)
            for c in range(TC[3]):
                wtp = loop_ps2.tile([128, 96], BF16, name="wtp", tag="wtp")
                nc.tensor.transpose(wtp[:, 0:32],
                                    w_att[96:128, c * 128:(c + 1) * 128],
                                    ident[96:128, 96:128],
                                    tile_position=(96, 0))
                nc.vector.tensor_copy(wts[:, c * BL + 3:c * BL + 4],
                                      wtp[:, 0:1])
                for dk in range(DC):
                    nc.tensor.matmul(
                        sc_ps[:, COFF + dk * BL + 3:COFF + dk * BL + 4],
                        enc_all[3][:, c * 512 + dk * 128:c * 512 + (dk + 1) * 128],
                        wts[:, c * BL + 3:c * BL + 4],
                        start=False, stop=(c == TC[3] - 1),
                        skip_group_check=True)

            nc.vector.tensor_copy(ctxT_sb[:], sc_ps[:, COFF:COFF + DC * BL])
            nc.vector.tensor_scalar(ctx_q[:], sc_ps[:, COFF:COFF + DC * BL],
                                    SX, None, ALU.mult)
            dst = cstk_d[:, bass.ds(t4, BL)].rearrange("(c p) b -> p c b", p=128)
            nc.sync.dma_start(dst, ctxT_sb[:])
            nc.vector.scalar_tensor_tensor(EnB[96:128, :], w1B[96:128, :],
                                           rseB[96:128, :], finv[96:128, 0:LB],
                                           ALU.mult, ALU.mult)
            nc.vector.tensor_tensor(accum_bf[96:128, 0:LB],
                                    accum_bf[96:128, 0:LB],
                                    EnB[96:128, :], ALU.add)
            nc.sync.dma_start(qd[0:1, 3 * 1024:4 * 1024],
                              accum_bf[96:97, :])
            nc.sync.dma_start(
                qbcB[:, 0:LB],
                qd[0:1, 3 * 1024:3 * 1024 + LB].partition_broadcast(128))

        assert UNROLL % 2 == 0, "gall parity needs even UNROLL"
        with tc.For_i(0, _nsteps * BL, UNROLL * BL,
                      hint_engines=(ET.PE, ET.Activation, ET.DVE, ET.SP)) as t4:
            for s in range(UNROLL):
                step_body(t4 + s * BL, s)

        if dbg_d is not None:
            nc.sync.dma_start(dbg2_d[:, 0:AC * BL], s_sb[:, :])
            nc.sync.dma_start(dbg_d[:, 0:1024], w_att[:, :])
            nc.sync.dma_start(dbg_d[:, 1024:1024 + HC * BL], h_bf[:, :])
            nc.sync.dma_start(dbg_d[:, 1056:1056 + TCmax * BL], wts[:, :])
            pass

        # ============ READOUT ============
        inner.close()
        post_sb = ctx.enter_context(tc.tile_pool(name="post_sb", bufs=1))
        post_st = ctx.enter_context(tc.tile_pool(name="post_st", bufs=2))
        post_ps = ctx.enter_context(tc.tile_pool(name="post_ps", bufs=2,
                                                 space="PSUM"))
        trash_holder.append(post_ps.tile([128, 128], BF16, name="trash_post"))

        xro = []
        for k in range(HC):
            tl = post_sb.tile([128, NS], BF16, name=f"xh{k}", tag=f"xh{k}")
            nc.sync.dma_start(tl[:], hstk_d[k * 128:(k + 1) * 128, :])
            xro.append(tl)
        for k in range(EC):
            tl = post_sb.tile([128, NS], BF16, name=f"xe{k}", tag=f"xe{k}")
            nc.sync.dma_start(tl[:], embT_d[k * 128:(k + 1) * 128, :])
            xro.append(tl)
        for k in range(DC):
            tl = post_sb.tile([128, NS], BF16, name=f"xc{k}", tag=f"xc{k}")
            nc.sync.dma_start(tl[:], cstk_d[k * 128:(k + 1) * 128, :])
            xro.append(tl)
        W_roe = [post_sb.tile([128, RO // 2], BF16, name=f"wre{k}", tag=f"wre{k}")
                 for k in range(XROC)]
        W_roo = [post_sb.tile([128, RO // 2], BF16, name=f"wro{k}", tag=f"wro{k}")
                 for k in range(XROC)]
        for k in range(XROC):
            nc.sync.dma_start(W_roe[k][:], W_roT_e_d[k * 128:(k + 1) * 128, :])
            nc.sync.dma_start(W_roo[k][:], W_roT_o_d[k * 128:(k + 1) * 128, :])
        b_ro_e = post_sb.tile([128, ROC], F32)
        nc.sync.dma_start(b_ro_e[:], b_ro_e_d[:, :])
        b_ro_o = post_sb.tile([128, ROC], F32)
        nc.sync.dma_start(b_ro_o[:], b_ro_o_d[:, :])
        b_out_col = post_sb.tile([128, VC], F32)
        nc.sync.dma_start(b_out_col[:], b_out_d[:, :])
        pe_touch(xro[0][:, 0:128])
        pe_touch(W_roe[0][:, 0:128])
        pe_touch(W_roo[0][:, 0:128])

        maxo = []
        for oc in range(ROC):
            Re = post_ps.tile([128, NS], F32, name="Re", tag="Re")
            for k in range(XROC):
                nc.tensor.matmul(Re[:], W_roe[k][:, oc * 128:(oc + 1) * 128],
                                 xro[k][:], start=(k == 0), stop=(k == XROC - 1))
            t1 = post_st.tile([128, NS], F32, name="t1", tag="t1")
            nc.scalar.activation(t1[:], Re[:], AF.Identity,
                                 bias=b_ro_e[:, oc:oc + 1], scale=1.0)
            Ro = post_ps.tile([128, NS], F32, name="Ro", tag="Re")
            for k in range(XROC):
                nc.tensor.matmul(Ro[:], W_roo[k][:, oc * 128:(oc + 1) * 128],
                                 xro[k][:], start=(k == 0), stop=(k == XROC - 1))
            t2 = post_st.tile([128, NS], F32, name="t2", tag="t2")
            nc.scalar.activation(t2[:], Ro[:], AF.Identity,
                                 bias=b_ro_o[:, oc:oc + 1], scale=1.0)
            mo = post_sb.tile([128, NS], BF16, name=f"mo{oc}", tag=f"mo{oc}")
            nc.vector.tensor_tensor(mo[:], t1[:], t2[:], ALU.max)
            maxo.append(mo)

        wo_pool = ctx.enter_context(tc.tile_pool(name="wo", bufs=6))
        first = True
        for vc in range(VC):
            wo = [wo_pool.tile([128, 128], BF16, name=f"wo{vc}_{k}", tag=f"wok{k}")
                  for k in range(ROC)]
            for k in range(ROC):
                nc.sync.dma_start(wo[k][:],
                                  W_outT_d[k * 128:(k + 1) * 128,
                                           vc * 128:(vc + 1) * 128])
            if first:
                pe_touch(wo[0][:, 0:128])
                pe_touch(maxo[0][:, 0:128])
                first = False
            L = post_ps.tile([128, NS], F32, name="L", tag="L")
            for k in range(ROC):
                nc.tensor.matmul(L[:], wo[k][:], maxo[k][:],
                                 start=(k == 0), stop=(k == ROC - 1))
            lo = post_st.tile([128, NS], F32, name="lo", tag="lo")
            nc.scalar.activation(lo[:], L[:], AF.Identity,
                                 bias=b_out_col[:, vc:vc + 1], scale=1.0)
            nc.sync.dma_start(out_d[vc * 128:(vc + 1) * 128, :], lo[:])

    return nc


def check_waits(nc, cap_note=""):
    bad = []
    for fn in nc.m.functions:
        for bb in fn.blocks:
            for inst in bb.instructions:
                c = inst.concise()
                nw = c.count("wait:")
                eng = c.split()[0] if c.split() else "?"
                if nw >= 2 and eng in ("PE", "ACT", "DVE", "PL"):
                    bad.append((nw, c[:180]))
    for nw, c in bad:
        print("WAITS", nw, c)
    return bad


def _prep_core(inputs, order, Tp, core):
    enc = np.asarray(inputs["encoder_outputs"], np.float32)
    labels = np.asarray(inputs["labels"])
    lens = np.asarray(inputs["enc_seq_len"], np.int64)
    embed = np.asarray(inputs["embed"], np.float32)

    bidx = [int(order[j * NCORE + core]) for j in range(BL)]
    m = {}
    for j in range(BL):
        b = bidx[j]
        ep = np.zeros((Tp[j], D), np.float32)
        ep[:T] = enc[b, :Tp[j] if Tp[j] <= T else T]
        m[f"enc_td{j}"] = _bf(ep)
        m[f"encT{j}"] = _bf(ep.T)
    emb = np.zeros((BL, S, E), np.float32)
    for j in range(BL):
        b = bidx[j]
        emb[j, 1:] = embed[labels[b, :S - 1].astype(np.int64)]
    embT = emb.transpose(2, 1, 0).reshape(E, NS)
    m["embT"] = _bf(embT)
    mask01 = np.zeros((BL, 1024), np.float32)
    for j in range(BL):
        mask01[j, :int(lens[bidx[j]])] = 1.0
    m["mask01"] = _bf(mask01)
    return m, bidx


def kernel(**inputs):
    lens = np.asarray(inputs["enc_seq_len"], np.int64)
    order = np.argsort(-lens, kind="stable")
    Tp = []
    for j in range(BL):
        mx = max(int(lens[order[j * NCORE + i]]) for i in range(NCORE))
        Tp.append(min(1024, ((mx + 127) // 128) * 128))

    perm = _gate_perm()
    W_ih = np.asarray(inputs["W_ih"], np.float32)[perm]
    W_hh = np.asarray(inputs["W_hh"], np.float32)[perm]
    b_sum = (np.asarray(inputs["b_ih"], np.float32)
             + np.asarray(inputs["b_hh"], np.float32))[perm]
    wfb = np.asarray(inputs["W_fb"], np.float32)[:, 0]
    wfb_safe = np.where(wfb >= 0, np.maximum(wfb, 1e-3),
                        np.minimum(wfb, -1e-3))
    shared = {
        "W_combT": _w8(np.concatenate([W_ih[:, E:].T, W_hh.T], 0)),
        "W_ih_embT": _bf(W_ih[:, :E].T),
        "W_encT": _bf(np.asarray(inputs["W_enc"], np.float32).T
                      / wfb_safe[None, :]),
        "W_sT": _bf(np.asarray(inputs["W_s"], np.float32).T),
        "wfert_col": _bf(np.asarray(inputs["W_fert"],
                                    np.float32).reshape(DC, 128).T),
        "vT_col": _bf(np.asarray(inputs["v_att"], np.float32).reshape(AC, 128).T),
        "wfb_colf": np.ascontiguousarray(
            wfb_safe.reshape(AC, 128).T.astype(np.float32)),
        "b_enc_col": np.ascontiguousarray(
            (np.asarray(inputs["b_enc"], np.float32) / wfb_safe)
            .reshape(AC, 128).T),
        "b_comb": np.ascontiguousarray(
            (b_sum * SG).reshape(GC, 128).T),
        "W_roT_e": _bf(np.asarray(inputs["W_ro"], np.float32)[0::2].T),
        "W_roT_o": _bf(np.asarray(inputs["W_ro"], np.float32)[1::2].T),
        "b_ro_e": np.ascontiguousarray(
            np.asarray(inputs["b_ro"], np.float32)[0::2].reshape(ROC, 128).T),
        "b_ro_o": np.ascontiguousarray(
            np.asarray(inputs["b_ro"], np.float32)[1::2].reshape(ROC, 128).T),
        "W_outT": _bf(np.asarray(inputs["W_out"], np.float32).T),
        "b_out_col": np.ascontiguousarray(
            np.asarray(inputs["b_out"], np.float32).reshape(VC, 128).T),
    }

    in_maps = []
    bidx_all = []
    for c in range(NCORE):
        m, bidx = _prep_core(inputs, order, Tp, c)
        m.update(shared)
        in_maps.append(m)
        bidx_all.append(bidx)

    nc = build_nc(Tp)
    nc.finalize()
    from concourse.bass_utils import run_bass_kernel_spmd
    trace = bool(os.environ.get("BASS_KERNEL_TRACE"))
    res = run_bass_kernel_spmd(nc, in_maps, core_ids=list(range(NCORE)),
                               trace=trace)
    global LAST_EXEC_NS, LAST_OUTS, LAST_META
    LAST_EXEC_NS = res.exec_time_ns
    outs = res.results
    LAST_OUTS = outs
    LAST_META = (order, Tp, bidx_all)

    logits = np.zeros((B, S, V), np.float32)
    for c in range(NCORE):
        o = outs[c]["out"].reshape(V, S, BL)
        for j in range(BL):
            logits[bidx_all[c][j]] = o[:, :, j].T
    return logits


if __name__ == "__main__":
    nc = build_nc([1024, 896, 768, 640])
    bad = check_waits(nc)
    print(f"{len(bad)} instructions with >=2 waits")


# revision 38
# speedup vs baseline: 1.3254x; 1.0264x over previous
"""Attention-LSTM decoder (B=32, T=1000, S=100, D=512, A=1024, H=1024,
E=640, V=10240, P=1024) on 8 trn2 NeuronCores.

Sharding: data-parallel over batch, 4 batches per core (one per "slot").
Batches are sorted by enc_seq_len; slot j holds ranks [j*8:(j+1)*8] so the
padded time extent Tp[j] (multiple of 128) is shared by all 8 cores and the
SPMD graph is identical across cores.

v3 design (vs. v2 baseline):
  - attention slots split into group A = slots {0,1,2} and group B = {3}.
    Group A's softmax / transpose / ctx-matvec tail executes underneath
    group B's tanh stretch; only B's short tail is serial.
  - tanh merged per (group, a-chunk): s_t/wfb is folded into the z-add via
    scalar_tensor_tensor with the per-partition scalar read directly from
    the s PSUM tile (host pre-divides W_s rows by wfb), so one ACT
    instruction covers all slots of a group.
  - softmax: exp -> one STT that applies the {0,1} mask AND emits the row
    sums via accum_out (no tensor_reduce); w*finv fused the same way.
  - gates accumulate in a single PSUM tile: W_hh part prefetched during the
    previous stretch, ctx part + xemb (via identity matmul) appended, and
    the activations read PSUM directly.
  - LSTM pointwise lowered to 5 STT + 1 TT + 1 TS using 2*sigmoid(x) =
    tanh(x/2) + 1; zoneout blends use pre-scaled c05/h05 computed during
    the previous stretch.
"""
import sys

sys.path.insert(0, "/opt/trn_rl_repo")

import os
import numpy as np
import ml_dtypes
from contextlib import ExitStack

import concourse.bass as bass
import concourse.tile as tile
import concourse.mybir as mybir
from concourse import bacc
from concourse.masks import make_identity

DT = mybir.dt
F32 = DT.float32
BF16 = DT.bfloat16
FP8 = DT.float8e4
AF = mybir.ActivationFunctionType
ALU = mybir.AluOpType
ET = mybir.EngineType

B, T, S = 32, 1000, 100
D, A, H, E, V, RO = 512, 1024, 1024, 640, 10240, 1024
ZH, ZC = 0.05, 0.15
NCORE = 8
BL = B // NCORE          # 4 batches (slots) per core
NS = S * BL              # 400 step-batch columns
GC = 4 * H // 128        # 32 gate chunks
HC = H // 128            # 8
AC = A // 128            # 8
DC = D // 128            # 4
EC = E // 128            # 5
ROC = RO // 2 // 128     # 4 chunks per maxout half
VC = V // 128            # 80 vocab chunks
XROC = (H + E + D) // 128  # 17 readout K-chunks

USE_FP8 = os.environ.get("KBFP8", "1") != "0"
SW = 64.0 if USE_FP8 else 1.0    # weight scale
SX = 16.0 if USE_FP8 else 1.0    # moving (h/ctx) scale
SG = SW * SX                      # psum scale for gates
WDT = FP8 if USE_FP8 else BF16

GA = (0, 1, 2)  # group A slots
GB = (3,)       # group B slots

bf16 = ml_dtypes.bfloat16
f8 = ml_dtypes.float8_e4m3
LAST_EXEC_NS = None
LAST_OUTS = None
LAST_META = None


def _bf(a):
    return np.ascontiguousarray(np.asarray(a, dtype=np.float32)).astype(bf16)


def _w8(a):
    a = np.asarray(a, dtype=np.float32) * SW
    return np.ascontiguousarray(a).astype(f8 if USE_FP8 else bf16)


# gate-permutation: reference gate order is [i|f|g|o]; we reorder rows to
# [i|f|o|g] so the three sigmoids are contiguous.
def _gate_perm():
    idx = np.arange(4 * H)
    return np.concatenate([idx[0:2 * H], idx[3 * H:4 * H], idx[2 * H:3 * H]])


def build_nc(Tp, debug=False):
    TC = [t // 128 for t in Tp]
    TCmax = max(TC)
    offA = [0, Tp[0], Tp[0] + Tp[1]]       # segment offsets in group-A tiles
    LA = Tp[0] + Tp[1] + Tp[2]
    LB = Tp[3]
    TpA = Tp[0]                            # group-A col extent (max of group)
    nc = bacc.Bacc("TRN2", target_bir_lowering=False)

    def param(name, shape, dt=BF16):
        return nc.declare_dram_parameter(name, list(shape), dt, isOutput=False)

    enc_td = [param(f"enc_td{j}", [Tp[j], D]) for j in range(BL)]
    encT = [param(f"encT{j}", [D, Tp[j]]) for j in range(BL)]
    embT_d = param("embT", [E, NS])
    W_combT_d = param("W_combT", [D + H, 4 * H], WDT)
    W_ih_embT_d = param("W_ih_embT", [E, 4 * H])
    W_encT_d = param("W_encT", [D, A])
    W_sT_d = param("W_sT", [H, A])                 # bf16 W_s.T
    wfert_col_d = param("wfert_col", [128, DC])
    vT_col_d = param("vT_col", [128, AC])
    wfb_colf_d = param("wfb_colf", [128, AC], F32)
    b_enc_col_d = param("b_enc_col", [128, AC], F32)
    b_comb_d = param("b_comb", [128, GC], F32)     # pre-scaled by SG on host
    mask01_d = param("mask01", [BL, 1024])         # {0,1} rows
    W_roT_e_d = param("W_roT_e", [H + E + D, RO // 2])
    W_roT_o_d = param("W_roT_o", [H + E + D, RO // 2])
    b_ro_e_d = param("b_ro_e", [128, ROC], F32)
    b_ro_o_d = param("b_ro_o", [128, ROC], F32)
    W_outT_d = param("W_outT", [RO // 2, V])
    b_out_d = param("b_out_col", [128, VC], F32)
    out_d = nc.declare_dram_parameter("out", [V, NS], F32, isOutput=True)

    qd = nc.dram_tensor("qd", [1, BL * 1024], BF16)
    dbg_d = nc.declare_dram_parameter("dbgt", [128, 1088], BF16,
                                      isOutput=True) \
        if os.environ.get("KBDBG") else None
    dbg2_d = nc.declare_dram_parameter("dbgt2", [128, 576], F32,
                                       isOutput=True) \
        if os.environ.get("KBDBG") else None
    hstk_d = nc.dram_tensor("hstk", [H, NS], BF16)
    cstk_d = nc.dram_tensor("cstk", [D, NS], BF16)
    xembT_d = nc.dram_tensor("xembT", [4 * H, NS + BL], BF16)

    with ExitStack() as ctx:
        tc = ctx.enter_context(tile.TileContext(nc))

        # ---------------- persistent pools ----------------
        persist = ctx.enter_context(tc.tile_pool(name="persist", bufs=1))
        ident = persist.tile([128, 128], BF16)
        make_identity(nc, ident[:])
        vT_col = persist.tile([128, AC], BF16)
        nc.sync.dma_start(vT_col[:], vT_col_d[:, :])
        wfb_colf = persist.tile([128, AC], F32)
        nc.sync.dma_start(wfb_colf[:], wfb_colf_d[:, :])
        wfert_col = persist.tile([128, DC], BF16)
        nc.sync.dma_start(wfert_col[:], wfert_col_d[:, :])
        b_enc_col = persist.tile([128, AC], F32)
        nc.sync.dma_start(b_enc_col[:], b_enc_col_d[:, :])
        b_comb = persist.tile([128, GC], F32)
        nc.sync.dma_start(b_comb[:], b_comb_d[:, :])
        mask01 = persist.tile([128, 1024], BF16)
        nc.vector.memset(mask01[:], 0.0)
        # col 0 = 1 on every row so dead-row softmax sums stay finite
        nc.vector.memset(mask01[:, 0:1], 1.0)
        for j in range(BL):
            nc.sync.dma_start(mask01[32 * j:32 * j + 1, :], mask01_d[j:j + 1, :])

        h_bf = persist.tile([128, HC * BL], BF16)
        h_q = persist.tile([128, HC * BL], WDT)
        c_st = persist.tile([128, HC * BL], F32)
        c05 = persist.tile([128, HC * BL], F32)
        h05 = persist.tile([128, HC * BL], F32)
        ctxT_sb = persist.tile([128, DC * BL], BF16)
        ctx_q = persist.tile([128, DC * BL], WDT)
        accum_bf = persist.tile([128, 1024], BF16)  # rows {0,32,64,96}
        w_att = persist.tile([128, 1024], BF16)
        finv = persist.tile([128, 1024], BF16)     # rows {0,32,64,96}, x0.5
        wts = persist.tile([128, TCmax * BL], BF16)
        s_sb = persist.tile([128, AC * BL], F32)
        zrow = persist.tile([1, 128], BF16)
        nc.vector.memset(zrow[:], 0.0)
        zpad = persist.tile([128, GC * BL], BF16)
        nc.vector.memset(zpad[:], 0.0)
        nc.sync.dma_start(
            xembT_d[:, NS:NS + BL].rearrange("(c p) b -> p c b", p=128),
            zpad[:])
        for t_ in (h_bf, h_q, c_st, c05, h05, ctxT_sb, ctx_q, accum_bf,
                   w_att, finv, wts):
            nc.vector.memset(t_[:], 0.0)

        inner = ctx.enter_context(ExitStack())
        e_pool = inner.enter_context(tc.tile_pool(name="e", bufs=1))
        e_A = [e_pool.tile([128, LA], BF16, name=f"eA{a}", tag=f"eA{a}")
               for a in range(AC)]
        e_B = [e_pool.tile([128, LB], BF16, name=f"eB{a}", tag=f"eB{a}")
               for a in range(AC)]

        trash_holder = []

        def pe_touch(ap):
            # phase-scoped trash tile (pre/post only; fp8 touches are no-ops)
            if ap.dtype not in (BF16,) or not trash_holder:
                return
            trash_ps = trash_holder[0]
            p = ap.shape[0]
            nc.tensor.transpose(trash_ps[0:min(ap.shape[1], 128), 0:p],
                                ap[:, 0:min(ap.shape[1], 128)], ident[0:p, 0:p])

        # ============ PRECOMPUTE PHASE ============
        with ExitStack() as pre:
            pre_sb = pre.enter_context(tc.tile_pool(name="pre_sb", bufs=1))
            pre_st = pre.enter_context(tc.tile_pool(name="pre_st", bufs=2))
            pre_ps = pre.enter_context(tc.tile_pool(name="pre_ps", bufs=1,
                                                    space="PSUM"))
            trash_holder.append(pre_ps.tile([128, 128], BF16, name="trash_pre"))

            W_encT = [pre_sb.tile([128, A], BF16, name=f"wenc{k}", tag=f"we{k}")
                      for k in range(DC)]
            for k in range(DC):
                nc.sync.dma_start(W_encT[k][:], W_encT_d[k * 128:(k + 1) * 128, :])
            pe_touch(W_encT[0][:, 0:128])

            for j in range(BL):
                ercs = [pre_st.tile([128, Tp[j]], BF16, name=f"erc{j}{k}",
                                    tag=f"erc{k}") for k in range(DC)]
                for k in range(DC):
                    nc.sync.dma_start(ercs[k][:], encT[j][k * 128:(k + 1) * 128, :])
                    pe_touch(ercs[k][:, 0:128])
                for a in range(AC):
                    pe2 = pre_ps.tile([128, 1024], F32, name="pe_e2", tag="pe_e2")
                    for k in range(DC):
                        for n0 in range(0, Tp[j], 512):
                            n1 = min(n0 + 512, Tp[j])
                            nc.tensor.matmul(pe2[:, n0:n1],
                                             W_encT[k][:, a * 128:(a + 1) * 128],
                                             ercs[k][:, n0:n1],
                                             start=(k == 0), stop=(k == DC - 1))
                    if j in GA:
                        dst = e_A[a][:, offA[j]:offA[j] + Tp[j]]
                    else:
                        dst = e_B[a][:, 0:Tp[j]]
                    nc.scalar.activation(dst, pe2[:, 0:Tp[j]],
                                         AF.Identity,
                                         bias=b_enc_col[:, a:a + 1], scale=1.0)
                pf = pre_ps.tile([1, 1024], F32, name="pf", tag="pf")
                for k in range(DC):
                    for n0 in range(0, Tp[j], 512):
                        n1 = min(n0 + 512, Tp[j])
                        nc.tensor.matmul(pf[0:1, n0:n1], wfert_col[:, k:k + 1],
                                         ercs[k][:, n0:n1],
                                         start=(k == 0), stop=(k == DC - 1))
                # finv = 0.5*sigmoid(x) = 0.25*tanh(0.5x) + 0.25  (no table sw)
                fstage = pre_st.tile([1, 1024], F32, name="fstage", tag="fstage")
                nc.scalar.activation(fstage[0:1, 0:Tp[j]], pf[0:1, 0:Tp[j]],
                                     AF.Tanh, scale=0.5)
                fst2 = pre_st.tile([1, 1024], BF16, name="fst2", tag="fst2")
                nc.vector.tensor_scalar(fst2[0:1, 0:Tp[j]], fstage[0:1, 0:Tp[j]],
                                        0.25, 0.25, ALU.mult, ALU.add)
                nc.sync.dma_start(finv[32 * j:32 * j + 1, 0:Tp[j]],
                                  fst2[0:1, 0:Tp[j]])

            embT_sb = [pre_sb.tile([128, NS], BF16, name=f"embs{k}", tag=f"em{k}")
                       for k in range(EC)]
            for k in range(EC):
                nc.sync.dma_start(embT_sb[k][:], embT_d[k * 128:(k + 1) * 128, :])
            W_ie = [pre_sb.tile([128, 4 * H], BF16, name=f"wie{k}", tag=f"wi{k}")
                    for k in range(EC)]
            for k in range(EC):
                nc.sync.dma_start(W_ie[k][:], W_ih_embT_d[k * 128:(k + 1) * 128, :])
            pe_touch(W_ie[0][:, 0:128])
            pe_touch(embT_sb[0][:, 0:128])
            for g in range(GC):
                px = pre_ps.tile([128, NS], F32, name="px", tag="pe_e2")
                for k in range(EC):
                    nc.tensor.matmul(px[:], W_ie[k][:, g * 128:(g + 1) * 128],
                                     embT_sb[k][:], start=(k == 0),
                                     stop=(k == EC - 1))
                # xemb scaled by SG, bias pre-scaled on host
                stg = pre_st.tile([128, NS], BF16, name="xstg", tag="xstg")
                nc.scalar.activation(stg[:], px[:], AF.Identity,
                                     bias=b_comb[:, g:g + 1], scale=SG)
                nc.sync.dma_start(xembT_d[g * 128:(g + 1) * 128, 0:NS], stg[:])

        trash_holder.clear()

        # ============ WEIGHTS (loop phase) ============
        ppsum = inner.enter_context(tc.tile_pool(name="ppsum", bufs=1,
                                                 space="PSUM"))
        Eps_A = ppsum.tile([128, TpA], F32)
        Eps_B = ppsum.tile([128, LB], F32)
        gall_tiles = [ppsum.tile([128, GC * BL], F32, name=f"gall{i}",
                                 tag=f"gall{i}") for i in range(2)]
        w_pool = inner.enter_context(tc.tile_pool(name="w", bufs=1))
        W_comb = [w_pool.tile([128, 4 * H], WDT, name=f"wc{k}", tag=f"wc{k}")
                  for k in range(DC + HC)]
        for k in range(DC + HC):
            nc.sync.dma_start(W_comb[k][:], W_combT_d[k * 128:(k + 1) * 128, :])
        W_sT = [w_pool.tile([128, A], BF16, name=f"ws{k}", tag=f"ws{k}")
                for k in range(HC)]
        for k in range(HC):
            nc.sync.dma_start(W_sT[k][:], W_sT_d[k * 128:(k + 1) * 128, :])

        qbcA = None  # set below (persistent, loop-carried)
        loop_sb = inner.enter_context(tc.tile_pool(name="lsb", bufs=2))
        loop_z = inner.enter_context(tc.tile_pool(name="lz", bufs=2))
        loop_q = inner.enter_context(tc.tile_pool(name="lq", bufs=2))
        loop_se = inner.enter_context(tc.tile_pool(name="lse", bufs=1))
        enc_pool = inner.enter_context(tc.tile_pool(name="encp", bufs=1))
        loop_ps2 = inner.enter_context(tc.tile_pool(name="lps2", bufs=1,
                                                    space="PSUM"))

        # enc resident in SBUF for the whole loop (identical every step)
        enc_all = []
        for j in range(BL):
            ea = enc_pool.tile([128, TC[j] * 512], BF16, name=f"ea{j}",
                               tag=f"ea{j}")
            nc.sync.dma_start(
                ea[:], enc_td[j][:, :].rearrange("(c p) d -> p c d", p=128))
            enc_all.append(ea)

        nc.vector.memset(Eps_A[:], 0.0)
        nc.vector.memset(Eps_B[:], 0.0)

        def emit_ghh(gall, g0, g1):
            # the gall tile is opened by a full-tile xet identity-matmul
            # (start=True); everything after accumulates onto written bytes.
            for g in range(g0, g1):
                for k in range(HC):
                    nc.tensor.matmul(gall[:, g * BL:(g + 1) * BL],
                                     W_comb[DC + k][:, g * 128:(g + 1) * 128],
                                     h_q[:, k * BL:(k + 1) * BL],
                                     start=False, stop=False,
                                     skip_group_check=True)

        # prologue: xemb(0) opens the psum tile, then W_hh part with h=0
        gall_cur = gall_tiles[0]
        xet0 = loop_sb.tile([128, GC * BL], BF16, name="xet", tag="xet")
        nc.sync.dma_start(xet0[:], xembT_d[:, 0:BL].rearrange(
            "(c p) b -> p c b", p=128))
        nc.tensor.matmul(gall_cur[:, 0:GC * BL], ident[:, :], xet0[:, :],
                         start=True, stop=False, skip_group_check=True)
        emit_ghh(gall_cur, 0, GC)
        # initial accum (=0) -> qd -> qbc (persistent loop-carried tiles)
        for j in range(BL):
            nc.sync.dma_start(qd[0:1, j * 1024:(j + 1) * 1024],
                              accum_bf[32 * j:32 * j + 1, :])
        qbcA = loop_q.tile([128, LA], BF16, name="qbcA", tag="qbcA")
        for ji, j in enumerate(GA):
            nc.sync.dma_start(
                qbcA[:, offA[ji]:offA[ji] + Tp[j]],
                qd[0:1, j * 1024:j * 1024 + Tp[j]].partition_broadcast(128))
        qbcB = loop_q.tile([128, LB], BF16, name="qbcB", tag="qbcB")
        nc.sync.dma_start(
            qbcB[:, 0:LB],
            qd[0:1, 3 * 1024:3 * 1024 + LB].partition_broadcast(128))

        # ============ STEP LOOP ============
        _nsteps = int(os.environ.get("KBSTEPS", S))
        UNROLL = int(os.environ.get("KBUNROLL", 10))

        last_sc = [None]
        scount = [0]

        def step_body(t4, par):
            gall = gall_tiles[par % 2]

            # ---- gates: ctx part (xemb + W_hh already accumulated) ----
            for g in range(GC):
                for k in range(DC):
                    nc.tensor.matmul(gall[:, g * BL:(g + 1) * BL],
                                     W_comb[k][:, g * 128:(g + 1) * 128],
                                     ctx_q[:, k * BL:(k + 1) * BL],
                                     start=False, stop=(g == GC - 1 and
                                                        k == DC - 1),
                                     skip_group_check=True)

            if dbg2_d is not None:
                sidx = scount[0]
                scount[0] += 1
                dtile = loop_sb.tile([128, GC * BL], F32, name="dtile",
                                     tag="dtile")
                nc.vector.tensor_copy(dtile[:], gall[:, :])
                nc.sync.dma_start(
                    dbg2_d[:, 64 + (sidx % 4) * 128:64 + (sidx % 4) * 128 + 128],
                    dtile[:])

            # ---- LSTM pointwise (fused) ----
            HB = HC * BL
            t_ifo = loop_sb.tile([128, 3 * HB], F32, name="t_ifo", tag="ti")
            nc.scalar.activation(t_ifo[:], gall[:, 0:3 * HB], AF.Tanh,
                                 scale=0.5 / SG)
            t_g = loop_sb.tile([128, HB], F32, name="t_g", tag="tg")
            nc.scalar.activation(t_g[:], gall[:, 3 * HB:], AF.Tanh,
                                 scale=1.0 / SG)
            av = loop_sb.tile([128, HB], F32, name="av", tag="av")
            nc.vector.scalar_tensor_tensor(av[:], t_ifo[:, HB:2 * HB], 1.0,
                                           c_st[:], ALU.add, ALU.mult)
            bv = loop_sb.tile([128, HB], F32, name="bv", tag="bv")
            nc.vector.scalar_tensor_tensor(bv[:], t_ifo[:, 0:HB], 1.0,
                                           t_g[:], ALU.add, ALU.mult)
            cn2 = loop_sb.tile([128, HB], F32, name="cn2", tag="cn2")
            nc.vector.tensor_tensor(cn2[:], av[:], bv[:], ALU.add)
            nc.vector.scalar_tensor_tensor(c_st[:], cn2[:], 0.5 * (1.0 - ZC),
                                           c05[:], ALU.mult, ALU.add)
            tcn = loop_sb.tile([128, HB], F32, name="tcn", tag="tcn")
            nc.scalar.activation(tcn[:], cn2[:], AF.Tanh, scale=0.5)
            hn2 = loop_sb.tile([128, HB], F32, name="hn2", tag="hn2")
            nc.vector.scalar_tensor_tensor(hn2[:], t_ifo[:, 2 * HB:3 * HB], 1.0,
                                           tcn[:], ALU.add, ALU.mult)
            nc.vector.scalar_tensor_tensor(h_bf[:], hn2[:], 0.5 * (1.0 - ZH),
                                           h05[:], ALU.mult, ALU.add)
            nc.vector.tensor_scalar(h_q[:], h_bf[:], SX, None, ALU.mult)
            dst = hstk_d[:, bass.ds(t4, BL)].rearrange("(c p) b -> p c b", p=128)
            nc.sync.dma_start(dst, h_bf[:])

            # ---- s' = h @ (W_s/wfb).T  (a-major so z can start early) ----
            # sc_ps holds s' in cols [0:32] and ctx in cols [32:48] (one
            # PSUM bank for both).
            COFF = AC * BL
            sc_ps = loop_ps2.tile([128, AC * BL + DC * BL], F32, name="sc_ps",
                                  tag="sc_ps")
            last_sc[0] = sc_ps
            nc.tensor.matmul(sc_ps[:, 0:COFF + DC * BL], zrow[0:1, 0:128],
                             zrow[0:1, 0:COFF + DC * BL],
                             start=True, stop=False, skip_group_check=True)
            for a in range(AC):
                for k in range(HC):
                    nc.tensor.matmul(sc_ps[:, a * BL:(a + 1) * BL],
                                     W_sT[k][:, a * 128:(a + 1) * 128],
                                     h_bf[:, k * BL:(k + 1) * BL],
                                     start=False,
                                     stop=(a == AC - 1 and k == HC - 1),
                                     skip_group_check=True)
                nc.vector.tensor_copy(s_sb[:, a * BL:(a + 1) * BL],
                                      sc_ps[:, a * BL:(a + 1) * BL])

            # ---- group A stretch ----
            for a in range(AC):
                z = loop_z.tile([128, LA], BF16, name="zA", tag="zA")
                for ji, j in enumerate(GA):
                    nc.vector.tensor_tensor(
                        z[:, offA[ji]:offA[ji] + Tp[j]],
                        e_A[a][:, offA[ji]:offA[ji] + Tp[j]],
                        qbcA[:, offA[ji]:offA[ji] + Tp[j]], ALU.add)
                tv = loop_z.tile([128, LA], BF16, name="tvA", tag="tvA")
                for ji, j in enumerate(GA):
                    nc.scalar.activation(tv[:, offA[ji]:offA[ji] + Tp[j]],
                                         z[:, offA[ji]:offA[ji] + Tp[j]],
                                         AF.Tanh,
                                         bias=s_sb[:, a * BL + j:a * BL + j + 1],
                                         scale=wfb_colf[:, a:a + 1])
                for ji, j in enumerate(GA):
                    kw = dict(start=(a == 0), stop=(a == AC - 1),
                              skip_group_check=True)
                    if j:
                        kw["tile_position"] = (0, 32 * j)
                    for n0 in range(0, Tp[j], 512):
                        n1 = min(n0 + 512, Tp[j])
                        nc.tensor.matmul(Eps_A[32 * j:32 * j + 1, n0:n1],
                                         vT_col[:, a:a + 1],
                                         tv[:, offA[ji] + n0:offA[ji] + n1],
                                         **kw)
                # interleave: xemb prefetch + W_hh part of NEXT step's gates
                if a == 0:
                    gnext = gall_tiles[(par + 1) % 2]
                    xet = loop_sb.tile([128, GC * BL], BF16, name="xet",
                                       tag="xet")
                    nc.sync.dma_start(xet[:], xembT_d[:, bass.ds(t4 + BL, BL)]
                                      .rearrange("(c p) b -> p c b", p=128))
                    nc.tensor.matmul(gnext[:, 0:GC * BL], ident[:, :],
                                     xet[:, :], start=True, stop=False,
                                     skip_group_check=True)
                else:
                    gnext = gall_tiles[(par + 1) % 2]
                emit_ghh(gnext, a * (GC // AC), (a + 1) * (GC // AC))
                if a == 0:
                    # zoneout prefactors for the next step
                    nc.vector.tensor_scalar(c05[:], c_st[:], ZC, None, ALU.mult)
                    nc.vector.tensor_scalar(h05[:], h_bf[:], ZH, None, ALU.mult)

            # ---- group A tail (overlaps group B stretch) ----
            EnA = loop_se.tile([128, TpA], BF16, name="EnA", tag="EnA")
            nc.scalar.activation(EnA[:], Eps_A[:], AF.Exp)
            w1A = loop_se.tile([128, TpA], BF16, name="w1A", tag="w1A")
            seA = loop_se.tile([128, 1], F32, name="seA", tag="seA")
            nc.vector.scalar_tensor_tensor(w1A[0:96, :], EnA[0:96, :], 1.0,
                                           mask01[0:96, 0:TpA],
                                           ALU.mult, ALU.mult,
                                           accum_out=seA[0:96, :])
            rseA = loop_se.tile([128, 1], F32, name="rseA", tag="rseA")
            nc.vector.reciprocal(rseA[0:96, :], seA[0:96, :])
            nc.vector.tensor_scalar(w_att[0:96, 0:TpA], w1A[0:96, :],
                                    rseA[0:96, :], None, ALU.mult)
            nc.vector.scalar_tensor_tensor(EnA[0:96, :], w1A[0:96, :],
                                           rseA[0:96, :], finv[0:96, 0:TpA],
                                           ALU.mult, ALU.mult)
            nc.vector.tensor_tensor(accum_bf[0:96, 0:TpA], accum_bf[0:96, 0:TpA],
                                    EnA[0:96, :], ALU.add)
            for j in GA:
                nc.sync.dma_start(qd[0:1, j * 1024:(j + 1) * 1024],
                                  accum_bf[32 * j:32 * j + 1, :])
            for ji, j in enumerate(GA):
                nc.sync.dma_start(
                    qbcA[:, offA[ji]:offA[ji] + Tp[j]],
                    qd[0:1, j * 1024:j * 1024 + Tp[j]].partition_broadcast(128))

            # ---- group B stretch, interleaved with group A transposes and
            #      ctx matvecs ----
            for a in range(AC):
                zb = loop_z.tile([128, LB], BF16, name="zB", tag="zB")
                nc.vector.tensor_tensor(zb[:, 0:LB], e_B[a][:, 0:LB],
                                        qbcB[:, 0:LB], ALU.add)
                tvb = loop_z.tile([128, LB], BF16, name="tvB", tag="tvB")
                nc.scalar.activation(tvb[:], zb[:], AF.Tanh,
                                     bias=s_sb[:, a * BL + 3:a * BL + 4],
                                     scale=wfb_colf[:, a:a + 1])
                kw = dict(start=(a == 0), stop=(a == AC - 1),
                          skip_group_check=True, tile_position=(0, 96))
                for n0 in range(0, LB, 512):
                    n1 = min(n0 + 512, LB)
                    nc.tensor.matmul(Eps_B[96:97, n0:n1], vT_col[:, a:a + 1],
                                     tvb[:, n0:n1], **kw)
                # group A transpose + ctx matvec for t-chunk c = a
                c = a
                if c < TC[0]:
                    wtp = loop_ps2.tile([128, 96], BF16, name="wtp", tag="wtp")
                    nc.tensor.transpose(wtp[:, 0:96],
                                        w_att[0:96, c * 128:(c + 1) * 128],
                                        ident[0:96, 0:96])
                    nsl = sum(1 for j in GA if TC[j] > c)
                    nc.vector.tensor_copy(wts[:, c * BL:c * BL + nsl],
                                          wtp[:, 0:32 * nsl:32])
                    for j in GA:
                        if c >= TC[j]:
                            continue
                        for dk in range(DC):
                            nc.tensor.matmul(
                                sc_ps[:, COFF + dk * BL + j:COFF + dk * BL + j + 1],
                                enc_all[j][:, c * 512 + dk * 128:
                                           c * 512 + (dk + 1) * 128],
                                wts[:, c * BL + j:c * BL + j + 1],
                                start=False, stop=(c == TC[j] - 1),
                                skip_group_check=True)

            # ---- group B tail (serial) ----
            EnB = loop_se.tile([128, LB], BF16, name="EnB", tag="EnB")
            nc.scalar.activation(EnB[:], Eps_B[:], AF.Exp)
            w1B = loop_se.tile([128, LB], BF16, name="w1B", tag="w1B")
            seB = loop_se.tile([128, 1], F32, name="seB", tag="seB")
            nc.vector.scalar_tensor_tensor(w1B[96:128, :], EnB[96:128, :], 1.0,
                                           mask01[96:128, 0:LB],
                                           ALU.mult, ALU.mult,
                                           accum_out=seB[96:128, :])
            rseB = loop_se.tile([128, 1], F32, name="rseB", tag="rseB")
            nc.vector.reciprocal(rseB[96:128, :], seB[96:128, :])
            nc.vector.tensor_scalar(w_att[96:128, 0:LB], w1B[96:128, :],
                                    rseB[96:128, :], None, ALU.mult)
            for c in range(TC[3]):
                wtp = loop_ps2.tile([128, 96], BF16, name="wtp", tag="wtp")
                nc.tensor.transpose(wtp[:, 0:32],
                                    w_att[96:128, c * 128:(c + 1) * 128],
                                    ident[96:128, 96:128],
                                    tile_position=(96, 0))
                nc.vector.tensor_copy(wts[:, c * BL + 3:c * BL + 4],
                                      wtp[:, 0:1])
                for dk in range(DC):
                    nc.tensor.matmul(
                        sc_ps[:, COFF + dk * BL + 3:COFF + dk * BL + 4],
                        enc_all[3][:, c * 512 + dk * 128:c * 512 + (dk + 1) * 128],
                        wts[:, c * BL + 3:c * BL + 4],
                        start=False, stop=(c == TC[3] - 1),
                        skip_group_check=True)

            nc.vector.tensor_copy(ctxT_sb[:], sc_ps[:, COFF:COFF + DC * BL])
            nc.vector.tensor_scalar(ctx_q[:], sc_ps[:, COFF:COFF + DC * BL],
                                    SX, None, ALU.mult)
            dst = cstk_d[:, bass.ds(t4, BL)].rearrange("(c p) b -> p c b", p=128)
            nc.sync.dma_start(dst, ctxT_sb[:])
            nc.vector.scalar_tensor_tensor(EnB[96:128, :], w1B[96:128, :],
                                           rseB[96:128, :], finv[96:128, 0:LB],
                                           ALU.mult, ALU.mult)
            nc.vector.tensor_tensor(accum_bf[96:128, 0:LB],
                                    accum_bf[96:128, 0:LB],
                                    EnB[96:128, :], ALU.add)
            nc.sync.dma_start(qd[0:1, 3 * 1024:4 * 1024],
                              accum_bf[96:97, :])
            nc.sync.dma_start(
                qbcB[:, 0:LB],
                qd[0:1, 3 * 1024:3 * 1024 + LB].partition_broadcast(128))

        assert UNROLL % 2 == 0, "gall parity needs even UNROLL"
        with tc.For_i(0, _nsteps * BL, UNROLL * BL,
                      hint_engines=(ET.PE, ET.Activation, ET.DVE, ET.SP)) as t4:
            for s in range(UNROLL):
                step_body(t4 + s * BL, s)

        if dbg_d is not None:
            nc.sync.dma_start(dbg2_d[:, 0:AC * BL], s_sb[:, :])
            nc.sync.dma_start(dbg_d[:, 0:1024], w_att[:, :])
            nc.sync.dma_start(dbg_d[:, 1024:1024 + HC * BL], h_bf[:, :])
            nc.sync.dma_start(dbg_d[:, 1056:1056 + TCmax * BL], wts[:, :])
            pass

        # ============ READOUT ============
        inner.close()
        post_sb = ctx.enter_context(tc.tile_pool(name="post_sb", bufs=1))
        post_st = ctx.enter_context(tc.tile_pool(name="post_st", bufs=2))
        post_ps = ctx.enter_context(tc.tile_pool(name="post_ps", bufs=2,
                                                 space="PSUM"))
        trash_holder.append(post_ps.tile([128, 128], BF16, name="trash_post"))

        xro = []
        for k in range(HC):
            tl = post_sb.tile([128, NS], BF16, name=f"xh{k}", tag=f"xh{k}")
            nc.sync.dma_start(tl[:], hstk_d[k * 128:(k + 1) * 128, :])
            xro.append(tl)
        for k in range(EC):
            tl = post_sb.tile([128, NS], BF16, name=f"xe{k}", tag=f"xe{k}")
            nc.sync.dma_start(tl[:], embT_d[k * 128:(k + 1) * 128, :])
            xro.append(tl)
        for k in range(DC):
            tl = post_sb.tile([128, NS], BF16, name=f"xc{k}", tag=f"xc{k}")
            nc.sync.dma_start(tl[:], cstk_d[k * 128:(k + 1) * 128, :])
            xro.append(tl)
        W_roe = [post_sb.tile([128, RO // 2], BF16, name=f"wre{k}", tag=f"wre{k}")
                 for k in range(XROC)]
        W_roo = [post_sb.tile([128, RO // 2], BF16, name=f"wro{k}", tag=f"wro{k}")
                 for k in range(XROC)]
        for k in range(XROC):
            nc.sync.dma_start(W_roe[k][:], W_roT_e_d[k * 128:(k + 1) * 128, :])
            nc.sync.dma_start(W_roo[k][:], W_roT_o_d[k * 128:(k + 1) * 128, :])
        b_ro_e = post_sb.tile([128, ROC], F32)
        nc.sync.dma_start(b_ro_e[:], b_ro_e_d[:, :])
        b_ro_o = post_sb.tile([128, ROC], F32)
        nc.sync.dma_start(b_ro_o[:], b_ro_o_d[:, :])
        b_out_col = post_sb.tile([128, VC], F32)
        nc.sync.dma_start(b_out_col[:], b_out_d[:, :])
        pe_touch(xro[0][:, 0:128])
        pe_touch(W_roe[0][:, 0:128])
        pe_touch(W_roo[0][:, 0:128])

        maxo = []
        for oc in range(ROC):
            Re = post_ps.tile([128, NS], F32, name="Re", tag="Re")
            for k in range(XROC):
                nc.tensor.matmul(Re[:], W_roe[k][:, oc * 128:(oc + 1) * 128],
                                 xro[k][:], start=(k == 0), stop=(k == XROC - 1))
            t1 = post_st.tile([128, NS], F32, name="t1", tag="t1")
            nc.scalar.activation(t1[:], Re[:], AF.Identity,
                                 bias=b_ro_e[:, oc:oc + 1], scale=1.0)
            Ro = post_ps.tile([128, NS], F32, name="Ro", tag="Re")
            for k in range(XROC):
                nc.tensor.matmul(Ro[:], W_roo[k][:, oc * 128:(oc + 1) * 128],
                                 xro[k][:], start=(k == 0), stop=(k == XROC - 1))
            t2 = post_st.tile([128, NS], F32, name="t2", tag="t2")
            nc.scalar.activation(t2[:], Ro[:], AF.Identity,
                                 bias=b_ro_o[:, oc:oc + 1], scale=1.0)
            mo = post_sb.tile([128, NS], BF16, name=f"mo{oc}", tag=f"mo{oc}")
            nc.vector.tensor_tensor(mo[:], t1[:], t2[:], ALU.max)
            maxo.append(mo)

        wo_pool = ctx.enter_context(tc.tile_pool(name="wo", bufs=6))
        first = True
        for vc in range(VC):
            wo = [wo_pool.tile([128, 128], BF16, name=f"wo{vc}_{k}", tag=f"wok{k}")
                  for k in range(ROC)]
            for k in range(ROC):
                nc.sync.dma_start(wo[k][:],
                                  W_outT_d[k * 128:(k + 1) * 128,
                                           vc * 128:(vc + 1) * 128])
            if first:
                pe_touch(wo[0][:, 0:128])
                pe_touch(maxo[0][:, 0:128])
                first = False
            L = post_ps.tile([128, NS], F32, name="L", tag="L")
            for k in range(ROC):
                nc.tensor.matmul(L[:], wo[k][:], maxo[k][:],
                                 start=(k == 0), stop=(k == ROC - 1))
            lo = post_st.tile([128, NS], F32, name="lo", tag="lo")
            nc.scalar.activation(lo[:], L[:], AF.Identity,
                                 bias=b_out_col[:, vc:vc + 1], scale=1.0)
            nc.sync.dma_start(out_d[vc * 128:(vc + 1) * 128, :], lo[:])

    return nc


def check_waits(nc, cap_note=""):
    bad = []
    for fn in nc.m.functions:
        for bb in fn.blocks:
            for inst in bb.instructions:
                c = inst.concise()
                nw = c.count("wait:")
                eng = c.split()[0] if c.split() else "?"
                if nw >= 2 and eng in ("PE", "ACT", "DVE", "PL"):
                    bad.append((nw, c[:180]))
    for nw, c in bad:
        print("WAITS", nw, c)
    return bad


def _prep_core(inputs, order, Tp, core):
    enc = np.asarray(inputs["encoder_outputs"], np.float32)
    labels = np.asarray(inputs["labels"])
    lens = np.asarray(inputs["enc_seq_len"], np.int64)
    embed = np.asarray(inputs["embed"], np.float32)

    bidx = [int(order[j * NCORE + core]) for j in range(BL)]
    m = {}
    for j in range(BL):
        b = bidx[j]
        ep = np.zeros((Tp[j], D), np.float32)
        ep[:T] = enc[b, :Tp[j] if Tp[j] <= T else T]
        m[f"enc_td{j}"] = _bf(ep)
        m[f"encT{j}"] = _bf(ep.T)
    emb = np.zeros((BL, S, E), np.float32)
    for j in range(BL):
        b = bidx[j]
        emb[j, 1:] = embed[labels[b, :S - 1].astype(np.int64)]
    embT = emb.transpose(2, 1, 0).reshape(E, NS)
    m["embT"] = _bf(embT)
    mask01 = np.zeros((BL, 1024), np.float32)
    for j in range(BL):
        mask01[j, :int(lens[bidx[j]])] = 1.0
    m["mask01"] = _bf(mask01)
    return m, bidx


def kernel(**inputs):
    lens = np.asarray(inputs["enc_seq_len"], np.int64)
    order = np.argsort(-lens, kind="stable")
    Tp = []
    for j in range(BL):
        mx = max(int(lens[order[j * NCORE + i]]) for i in range(NCORE))
        Tp.append(min(1024, ((mx + 127) // 128) * 128))

    perm = _gate_perm()
    W_ih = np.asarray(inputs["W_ih"], np.float32)[perm]
    W_hh = np.asarray(inputs["W_hh"], np.float32)[perm]
    b_sum = (np.asarray(inputs["b_ih"], np.float32)
             + np.asarray(inputs["b_hh"], np.float32))[perm]
    wfb = np.asarray(inputs["W_fb"], np.float32)[:, 0]
    wfb_safe = np.where(wfb >= 0, np.maximum(wfb, 1e-3),
                        np.minimum(wfb, -1e-3))
    shared = {
        "W_combT": _w8(np.concatenate([W_ih[:, E:].T, W_hh.T], 0)),
        "W_ih_embT": _bf(W_ih[:, :E].T),
        "W_encT": _bf(np.asarray(inputs["W_enc"], np.float32).T
                      / wfb_safe[None, :]),
        "W_sT": _bf(np.asarray(inputs["W_s"], np.float32).T),
        "wfert_col": _bf(np.asarray(inputs["W_fert"],
                                    np.float32).reshape(DC, 128).T),
        "vT_col": _bf(np.asarray(inputs["v_att"], np.float32).reshape(AC, 128).T),
        "wfb_colf": np.ascontiguousarray(
            wfb_safe.reshape(AC, 128).T.astype(np.float32)),
        "b_enc_col": np.ascontiguousarray(
            (np.asarray(inputs["b_enc"], np.float32) / wfb_safe)
            .reshape(AC, 128).T),
        "b_comb": np.ascontiguousarray(
            (b_sum * SG).reshape(GC, 128).T),
        "W_roT_e": _bf(np.asarray(inputs["W_ro"], np.float32)[0::2].T),
        "W_roT_o": _bf(np.asarray(inputs["W_ro"], np.float32)[1::2].T),
        "b_ro_e": np.ascontiguousarray(
            np.asarray(inputs["b_ro"], np.float32)[0::2].reshape(ROC, 128).T),
        "b_ro_o": np.ascontiguousarray(
            np.asarray(inputs["b_ro"], np.float32)[1::2].reshape(ROC, 128).T),
        "W_outT": _bf(np.asarray(inputs["W_out"], np.float32).T),
        "b_out_col": np.ascontiguousarray(
            np.asarray(inputs["b_out"], np.float32).reshape(VC, 128).T),
    }

    in_maps = []
    bidx_all = []
    for c in range(NCORE):
        m, bidx = _prep_core(inputs, order, Tp, c)
        m.update(shared)
        in_maps.append(m)
        bidx_all.append(bidx)

    nc = build_nc(Tp)
    nc.finalize()
    from concourse.bass_utils import run_bass_kernel_spmd
    trace = bool(os.environ.get("BASS_KERNEL_TRACE"))
    res = run_bass_kernel_spmd(nc, in_maps, core_ids=list(range(NCORE)),
                               trace=trace)
    global LAST_EXEC_NS, LAST_OUTS, LAST_META
    LAST_EXEC_NS = res.exec_time_ns
    outs = res.results
    LAST_OUTS = outs
    LAST_META = (order, Tp, bidx_all)

    logits = np.zeros((B, S, V), np.float32)
    for c in range(NCORE):
        o = outs[c]["out"].reshape(V, S, BL)
        for j in range(BL):
            logits[bidx_all[c][j]] = o[:, :, j].T
    return logits


if __name__ == "__main__":
    nc = build_nc([1024, 896, 768, 640])
    bad = check_waits(nc)
    print(f"{len(bad)} instructions with >=2 waits")
